# revision 17
# baseline (speedup 1.0000x reference)
"""Trainium2 Bass kernel for a 3-layer GIN-style GNN (nn_BaseGNN).

Sharding: data-parallel over nodes/edges by dst-owner across 8 NeuronCores.
Aggregation = one-hot matmuls over 128-edge chunks (PSUM accumulate), edge
source rows fetched from a replicated table in DRAM via gpsimd dma_gather.

Key structure (v2):
 - Layer 0 aggregates raw x (128-dim) and applies the encoder afterwards by
   linearity: z0 = (x_own + sum_j x_j) @ W_enc + (1+deg) b_enc.  The x table
   is a static replicated input; no initial table build or AllGather.
 - BatchNorm is algebraically folded: with s = gamma/sigma > 0,
   h = relu(s*(z - shift)) = s * relu(z - shift), so the inter-layer tables
   store RAW z2.  relu(z - shift) = max(z, shift) - shift; the max is applied
   per gathered tile, and the -shift*deg rank-1 term is injected into the
   aggregation PSUM via a 1-partition matmul with the in-degree vector.
   s is folded into the next layer's W1 (per-partition scale) and into the
   pooling output.  Hence slice writes (raw z2 transposes) do not wait for
   the BN stats AllReduce, which overlaps the table AllGathers.
 - Pooling of layer l runs inside layer l+1 (after global stats arrive),
   off the critical path.  The last layer keeps stats AllReduce + pool.

All instruction streams are identical across cores (SPMD); per-core
variation lives exclusively in input data.
"""

import numpy as np
import ml_dtypes

import concourse.bass as bass
import concourse.bacc as bacc
import concourse.mybir as mybir
import concourse.tile as tile
from concourse.masks import make_identity

BF16 = mybir.dt.bfloat16
FP32 = mybir.dt.float32
I16 = mybir.dt.int16
FP32R = mybir.dt.float32r
AF = mybir.ActivationFunctionType
ALU = mybir.AluOpType

N_CORES = 8
GI_CHUNKS = 8           # 128-edge chunks per dma_gather instruction
                        # (1024 idxs = 1024 descs fits one SWDGE queue ring;
                        # bigger gathers block gpsimd on their own drain)
CB = 8                 # one-hot chunks per batched is_equal
N_SWDGE_QUEUES = 4
BN_EPS = 1e-5


def _cdiv(a, b):
    return (a + b - 1) // b


class Plan:
    pass


# ==================================================================== host
def preprocess(x, edge_index, batch, num_graphs, W_enc, b_enc, W1, b1, W2, b2,
               gamma, beta, W_fc1, b_fc1, W_fc2, b_fc2):
    p = Plan()
    N, F_IN = x.shape
    D = W_enc.shape[1]
    L = W1.shape[0]
    G = int(num_graphs)
    E = edge_index.shape[1]
    C = N_CORES
    assert N % C == 0
    NC = N // C
    assert NC % 2 == 0
    HALF = NC // 2
    W = _cdiv(NC, 128)
    p.N, p.F_IN, p.D, p.L, p.G, p.E = N, F_IN, D, L, G, E
    p.NC, p.HALF, p.W = NC, HALF, W
    assert D == 256 and F_IN == 128, "layout hardcodes D=256, F_IN=128"
    assert HALF < 32768, "int16 gather index range"
    assert np.all(np.asarray(gamma) > 0), "BN fold requires gamma > 0"

    src = np.asarray(edge_index[0], np.int64)
    dst = np.asarray(edge_index[1], np.int64)
    batch = np.asarray(batch, np.int64)

    owner = dst // NC
    src_owner = src // NC
    src_local = src % NC
    src_half = (src_local >= HALF).astype(np.int64)
    table_row = HALF * src_owner + (src_local % HALF)
    dst_local = dst - owner * NC
    win = dst_local // 128

    counts = np.zeros((C, 2, W), np.int64)
    np.add.at(counts, (owner, src_half, win), 1)
    k_fix = _cdiv(counts, 128).max(axis=0)          # [2, W]
    p.k_fix = k_fix
    K_pass = k_fix.sum(axis=1).astype(np.int64)
    p.K_pass = K_pass
    p.Ktot = int(K_pass.sum())

    order = np.lexsort((dst_local, win, src_half, owner))
    so_owner = owner[order]
    so_half = src_half[order]
    so_win = win[order]
    so_row = table_row[order]
    so_dstloc = (dst_local - win * 128)[order]

    # chunk slot base per (p, w) in each pass stream
    slot_base = np.zeros((2, W), np.int64)
    for ph in range(2):
        b = 0
        for w in range(W):
            slot_base[ph, w] = b
            b += int(k_fix[ph, w]) * 128

    # per-(c,p,w) edge segment boundaries in the sorted arrays
    seg = np.zeros((C, 2, W, 2), np.int64)
    keys = ((so_owner * 2 + so_half) * W + so_win)
    bounds = np.searchsorted(keys, np.arange(C * 2 * W + 1))
    for c in range(C):
        for ph in range(2):
            for w in range(W):
                kk = (c * 2 + ph) * W + w
                seg[c, ph, w] = bounds[kk], bounds[kk + 1]

    def wrap16(lin):
        S = lin.shape[0] // 16
        t = lin.reshape(S, 16).T
        return np.ascontiguousarray(np.tile(t, (8, 1)).astype(np.int16))

    cnt = np.bincount(batch, minlength=G).astype(np.float64)
    cnt_inv = (1.0 / np.maximum(cnt, 1.0)).astype(np.float32)

    WPAD = W * 128
    xf = np.asarray(x, np.float32)
    xb_full = xf.astype(ml_dtypes.bfloat16).reshape(C, 2, HALF, F_IN)
    xtabs_bf = [np.ascontiguousarray(xb_full[:, 0].reshape(C * HALF, F_IN)),
                np.ascontiguousarray(xb_full[:, 1].reshape(C * HALF, F_IN))]
    dst_iota = np.arange(128, dtype=np.float32)
    p.per_core = []
    for c in range(C):
        d = {}
        for ph in range(2):
            K = int(K_pass[ph])
            idx = np.zeros((K * 128,), np.int16)
            dl = np.full((K * 128,), -1.0, np.float32)
            for w in range(W):
                a, b = seg[c, ph, w]
                n = b - a
                sb = int(slot_base[ph, w])
                idx[sb:sb + n] = so_row[a:b].astype(np.int16)
                dl[sb:sb + n] = so_dstloc[a:b].astype(np.float32)
            d[f"idx{ph}"] = wrap16(idx)
            # host-pregathered x rows for layer 0: [128 slot, K, 128 feat]
            xg = np.take(xtabs_bf[ph], idx.reshape(K, 128).astype(np.int64),
                         axis=0)                      # [K, 128, F_IN]
            d[f"xg{ph}"] = np.ascontiguousarray(
                xg.transpose(1, 0, 2).reshape(128, K * F_IN))
            if ph == 0:
                dl0 = dl
            else:
                dl = np.concatenate([dl0, dl])
        dlm = dl.reshape(p.Ktot, 128).T               # [128 slot, Ktot]
        # precomputed one-hot stream: oh[p, c, d] = (dstloc[p,c] == d)
        d["ohs"] = np.ascontiguousarray(
            (dlm[:, :, None] == dst_iota[None, None, :])
            .astype(ml_dtypes.bfloat16).reshape(128, p.Ktot * 128))
        nb = batch[c * NC:(c + 1) * NC]
        g_lo = int(nb[0])
        span = int(nb[-1]) - g_lo + 1
        assert span <= 128, f"core {c} spans {span} graphs"
        bl = np.full((W * 128,), -1.0, np.float32)
        bl[:NC] = (nb - g_lo).astype(np.float32)
        blm = bl.reshape(W, 128).T                    # [128 node, W]
        # pooling one-hot const: pooh[p, j, d] = (batchloc[p,j] == d)
        d["pooh"] = np.ascontiguousarray(
            (blm[:, :, None] == dst_iota[None, None, :])
            .astype(ml_dtypes.bfloat16).reshape(128, W * 128))
        pl = np.zeros((128, 512), np.float32)
        hi = min(128, G - g_lo)
        pl[np.arange(hi), g_lo + np.arange(hi)] = cnt_inv[g_lo:g_lo + hi]
        d["placem"] = pl.astype(ml_dtypes.bfloat16)
        d["xTown"] = np.ascontiguousarray(
            xf[c * NC:(c + 1) * NC].T).astype(ml_dtypes.bfloat16)
        degc = np.bincount(dst_local[owner == c], minlength=NC).astype(np.float32)
        dv = np.zeros((1, WPAD), np.float32)
        dv[0, :NC] = degc
        d["deg"] = dv.astype(ml_dtypes.bfloat16)
        p.per_core.append(d)

    sh = {}
    sh["wenc"] = np.asarray(W_enc, np.float32).astype(ml_dtypes.bfloat16)
    w1c = np.zeros((128, L * 4 * 128), np.float32)
    w2c = np.zeros((128, L * 4 * 128), np.float32)
    for l in range(L):
        for k in range(2):
            for m in range(2):
                col = ((l * 2 + k) * 2 + m) * 128
                w1c[:, col:col + 128] = W1[l, 128 * k:128 * (k + 1), 128 * m:128 * (m + 1)]
                w2c[:, col:col + 128] = W2[l, 128 * k:128 * (k + 1), 128 * m:128 * (m + 1)]
    sh["w1"] = w1c
    sh["w2"] = w2c

    def col2(v):
        return np.ascontiguousarray(np.asarray(v, np.float32).reshape(2, 128).T)

    sh["bencc"] = col2(b_enc)
    sh["bencrow"] = np.asarray(b_enc, np.float32).reshape(1, 256).astype(
        ml_dtypes.bfloat16)
    p.benc_nonzero = bool(np.any(np.asarray(b_enc) != 0))
    sh["b1"] = np.concatenate([col2(b1[l]) for l in range(L)], axis=1)
    sh["b2"] = np.concatenate([col2(b2[l]) for l in range(L)], axis=1)
    sh["gaminv"] = np.concatenate(
        [col2(1.0 / np.asarray(gamma[l], np.float64)) for l in range(L)], axis=1)
    sh["bet"] = np.concatenate([col2(beta[l]) for l in range(L)], axis=1)
    DL = D * L
    sh["wfc1"] = np.ascontiguousarray(
        np.asarray(W_fc1, np.float32).reshape(DL // 128, 128, 128)
        .transpose(1, 0, 2).reshape(128, DL))
    sh["wfc2"] = np.asarray(W_fc2, np.float32).reshape(128, 1)
    sh["bfc1"] = np.asarray(b_fc1, np.float32).reshape(128, 1)
    sh["bfc2"] = np.asarray(b_fc2, np.float32).reshape(1, 1)
    p.shared = sh
    return p


# ================================================================= program
def build_program(p, reps=0, skip_coll=False, dbg=False):
    C = N_CORES
    N, D, L, W, NC, HALF = p.N, p.D, p.L, p.W, p.NC, p.HALF
    NROW = C * HALF
    K_pass, k_fix, Ktot = p.K_pass, p.k_fix, p.Ktot
    NCH = _cdiv(NC, 128)
    MJ = _cdiv(NC, 512)
    DL = D * L
    WPAD = W * 128

    nc = bacc.Bacc("TRN2", target_bir_lowering=False, debug=False,
                   enable_asserts=False, num_devices=C,
                   num_swdge_queues=N_SWDGE_QUEUES)

    ein = {}

    def EIN(name, shape, dt):
        ein[name] = nc.dram_tensor(name, list(shape), dt, kind="ExternalInput").ap()
        return ein[name]

    xTown = EIN("xTown", [128, NC], BF16)
    idx_in = [EIN("idx0", [128, int(K_pass[0]) * 8], I16),
              EIN("idx1", [128, int(K_pass[1]) * 8], I16)]
    xg_in = [EIN("xg0", [128, int(K_pass[0]) * 128], BF16),
             EIN("xg1", [128, int(K_pass[1]) * 128], BF16)]
    ohs_in = EIN("ohs", [128, Ktot * 128], BF16)
    pooh_in = EIN("pooh", [128, W * 128], BF16)
    placem_in = EIN("placem", [128, 512], BF16)
    deg_in = EIN("deg", [1, WPAD], BF16)
    wenc_in = EIN("wenc", [128, 256], BF16)
    w1_in = EIN("w1", [128, L * 4 * 128], FP32R)
    w2_in = EIN("w2", [128, L * 4 * 128], FP32R)
    bencc_in = EIN("bencc", [128, 2], FP32)
    bencrow_in = EIN("bencrow", [1, 256], BF16)
    b1_in = EIN("b1", [128, L * 2], FP32)
    b2_in = EIN("b2", [128, L * 2], FP32)
    gaminv_in = EIN("gaminv", [128, L * 2], FP32)
    bet_in = EIN("bet", [128, L * 2], FP32)
    wfc1_in = EIN("wfc1", [128, DL], FP32)
    wfc2_in = EIN("wfc2", [128, 1], FP32)
    bfc1_in = EIN("bfc1", [128, 1], FP32)
    bfc2_in = EIN("bfc2", [1, 1], FP32)

    out_d = nc.dram_tensor("out", [1, 512], FP32, kind="ExternalOutput").ap()
    dbg_t = {}
    if dbg:
        for nm, shape, dt_ in [
                ("d_xsum", [128, NC], BF16), ("d_zpre0", [128, 2 * NC], FP32),
                ("d_z20", [128, 2 * NC], BF16), ("d_u0", [128, 2 * NC], BF16),
                ("d_mrep", [128, 256], BF16), ("d_zpre1", [128, 2 * NC], FP32),
                ("d_z21", [128, 2 * NC], BF16), ("d_zpre2", [128, 2 * NC], FP32)]:
            dbg_t[nm] = nc.dram_tensor(nm, shape, dt_,
                                       kind="ExternalOutput").ap()

    tableL = nc.dram_tensor("tableL", [NROW, 256], BF16, kind="Internal").ap()
    tableH = nc.dram_tensor("tableH", [NROW, 256], BF16, kind="Internal").ap()
    sliceT = [nc.dram_tensor(f"slice{h}", [HALF, 256], BF16, kind="Internal").ap()
              for h in range(2)]
    arbn_in = nc.dram_tensor("arbn_in", [128, 4], FP32, kind="Internal").ap()
    arbn_out = nc.dram_tensor("arbn_out", [128, 4], FP32, kind="Internal",
                              addr_space="Shared").ap()
    arp_in = nc.dram_tensor("arp_in", [DL, 512], FP32, kind="Internal").ap()
    arp_out = nc.dram_tensor("arp_out", [DL, 512], FP32, kind="Internal",
                             addr_space="Shared").ap()

    RG = [list(range(C))]
    half_t = [tableL, tableH]

    import contextlib
    with tile.TileContext(nc) as tc, contextlib.ExitStack() as ctx:
        consts = ctx.enter_context(tc.tile_pool(name="consts", bufs=1))
        gpool_s = ctx.enter_context(tc.tile_pool(name="gP", bufs=6))
        gpool = [gpool_s, gpool_s]
        ohpool = ctx.enter_context(tc.tile_pool(name="oh", bufs=2))
        zpool = ctx.enter_context(tc.tile_pool(name="z", bufs=1))
        spool = ctx.enter_context(tc.tile_pool(name="s", bufs=2))
        tpool = ctx.enter_context(tc.tile_pool(name="t", bufs=2))
        ppool = ctx.enter_context(tc.tile_pool(name="prm", bufs=1))
        ps_agg = ctx.enter_context(tc.tile_pool(name="ps_agg", bufs=2, space="PSUM"))
        ps_mlp = ctx.enter_context(tc.tile_pool(name="ps_mlp", bufs=2, space="PSUM"))
        ps_msc = ctx.enter_context(tc.tile_pool(name="ps_msc", bufs=2, space="PSUM"))

        def load_const(apin, shape, dt):
            t = consts.tile(shape, dt, name="c_" + apin.tensor.name)
            nc.sync.dma_start(t[:], apin[:])
            return t

        pooh_sb = load_const(pooh_in, [128, W * 128], BF16)
        wenc_sb = load_const(wenc_in, [128, 256], BF16)
        w1_sb = load_const(w1_in, [128, L * 4 * 128], FP32R)
        w2_sb = load_const(w2_in, [128, L * 4 * 128], FP32R)
        bencc_sb = load_const(bencc_in, [128, 2], FP32)
        bencrow_sb = load_const(bencrow_in, [1, 256], BF16)
        b1_sb = load_const(b1_in, [128, L * 2], FP32)
        b2_sb = load_const(b2_in, [128, L * 2], FP32)
        gaminv_sb = load_const(gaminv_in, [128, L * 2], FP32)
        bet_sb = load_const(bet_in, [128, L * 2], FP32)
        wfc1_sb = load_const(wfc1_in, [128, DL], FP32)
        wfc2_sb = load_const(wfc2_in, [128, 1], FP32)
        bfc1_sb = load_const(bfc1_in, [128, 1], FP32)
        bfc2_sb = load_const(bfc2_in, [1, 1], FP32)
        placem_sb = load_const(placem_in, [128, 512], BF16)
        deg_sb = load_const(deg_in, [1, WPAD], BF16)
        idx_sb = [load_const(idx_in[0], [128, int(K_pass[0]) * 8], I16),
                  load_const(idx_in[1], [128, int(K_pass[1]) * 8], I16)]
        ident = consts.tile([128, 128], BF16)
        make_identity(nc, ident[:])
        ones_col = consts.tile([1, 128], BF16)
        nc.vector.memset(ones_col[:], 1.0)

        n_ginst = [int(_cdiv(int(K_pass[ph]), GI_CHUNKS)) for ph in range(2)]

        def emit_body():
            # ---------- gathers helper ------------------------------------
            def emit_gathers(ph, table_ap, es):
                tiles = []
                K = int(K_pass[ph])
                for i in range(n_ginst[ph]):
                    c0 = i * GI_CHUNKS
                    nch = min(GI_CHUNKS, K - c0)
                    g = gpool[ph].tile([128, GI_CHUNKS, es], BF16, tag="g",
                                       name=f"g{ph}t")
                    nc.gpsimd.dma_gather(
                        g[:, :nch, :], table_ap[:], idx_sb[ph][:, c0 * 8:(c0 + nch) * 8],
                        num_idxs=nch * 128, num_idxs_reg=nch * 128, elem_size=es,
                        single_packet=False, queue_num=(i % N_SWDGE_QUEUES),
                    )
                    tiles.append(g)
                return tiles

            class GetG:
                """Lazily applies max(g, shift) (u-space relu prep) the first
                time a gather tile is touched, in consumption order, so the
                DVE stream order matches the window loops (no queue cycle)."""
                def __init__(self, ph, tiles, mrep):
                    self.ph, self.tiles, self.mrep = ph, tiles, mrep
                    self.maxed = set()

                def __call__(self, ci):
                    gi = ci // GI_CHUNKS
                    g = self.tiles[gi]
                    if self.mrep is not None and gi not in self.maxed:
                        nch = min(GI_CHUNKS, int(K_pass[self.ph]) - gi * GI_CHUNKS)
                        nc.vector.tensor_tensor(
                            g[:, :nch, :], g[:, :nch, :],
                            self.mrep[:].unsqueeze(1).to_broadcast([128, nch, 256]),
                            op=ALU.max)
                        self.maxed.add(gi)
                    return g[:, ci % GI_CHUNKS, :]

            class OhLazy:
                """Loads precomputed one-hot groups on first touch, in
                consumption order.  base = ohs column of this pass's chunk 0."""
                def __init__(self, base, K):
                    self.base, self.K = base, K
                    self.map = {}

                def __call__(self, ci):
                    if ci not in self.map:
                        g0 = ci - ci % CB
                        cn = min(CB, self.K - g0)
                        oh = ohpool.tile([128, CB * 128], BF16, tag="oh",
                                         name="oht")
                        c0 = self.base + g0
                        nc.scalar.dma_start(
                            oh[:, :cn * 128],
                            ohs_in[:, c0 * 128:(c0 + cn) * 128])
                        for k in range(cn):
                            self.map[g0 + k] = (oh, k * 128)
                    return self.map[ci]

            # ---------- shared MLP (z_pre -> z2), feature-major ------------
            def mlp(l, z_pre, z2, w1src, w1cols):
                for jj in range(MJ):
                    lo = jj * 512
                    nw = min(512, NC - lo)
                    z1t = [tpool.tile([128, 512], FP32R, tag=f"z1_{m}", bufs=1,
                                      name=f"z1t{m}") for m in range(2)]
                    for m in range(2):
                        ps = ps_mlp.tile([128, 512], FP32, tag="mlp")
                        for k in range(2):
                            col = w1cols(k, m)
                            nc.tensor.matmul(ps[:, :nw],
                                             w1src[:, col:col + 128],
                                             z_pre[k][:, lo:lo + nw],
                                             start=(k == 0), stop=(k == 1))
                        nc.scalar.activation(z1t[m][:, :nw], ps[:, :nw], AF.Relu,
                                             bias=b1_sb[:, 2 * l + m:2 * l + m + 1])
                    for m in range(2):
                        ps = ps_mlp.tile([128, 512], FP32, tag="mlp")
                        for k in range(2):
                            col = ((l * 2 + k) * 2 + m) * 128
                            nc.tensor.matmul(ps[:, :nw],
                                             w2_sb[:, col:col + 128],
                                             z1t[k][:, :nw],
                                             start=(k == 0), stop=(k == 1))
                        nc.scalar.activation(z2[m][:, lo:lo + nw], ps[:, :nw],
                                             AF.Identity,
                                             bias=b2_sb[:, 2 * l + m:2 * l + m + 1])

            # ---------- BN stats of z2 -> small AllReduce ------------------
            def stats_ar(z2):
                nbch = _cdiv(NC, 512)
                stt = spool.tile([128, 4], FP32, tag="stt")
                bnacc = tpool.tile([128, nbch, 6], FP32, tag="bnacc", bufs=1)
                for m in range(2):
                    for jj in range(nbch):
                        lo = jj * 512
                        nw = min(512, NC - lo)
                        nc.vector.bn_stats(bnacc[:, jj, :], z2[m][:, lo:lo + nw])
                    ag = spool.tile([128, 2], FP32, tag="bnag")
                    nc.vector.bn_aggr(ag[:], bnacc[:])
                    sq = spool.tile([128, 1], FP32, tag="bnsq")
                    nc.vector.tensor_tensor(sq[:], ag[:, 0:1], ag[:, 0:1],
                                            op=ALU.mult)
                    nc.vector.tensor_copy(stt[:, 2 * m:2 * m + 1], ag[:, 0:1])
                    nc.vector.tensor_tensor(stt[:, 2 * m + 1:2 * m + 2], ag[:, 1:2],
                                            sq[:], op=ALU.add)
                nc.sync.dma_start(arbn_in[:], stt[:])
                if not skip_coll:
                    nc.gpsimd.collective_compute(
                        "AllReduce", ALU.add, replica_groups=RG,
                        ins=[arbn_in.opt()], outs=[arbn_out.opt()])

            # ---------- stats -> scl / shift / rows / m_rep / w1s ----------
            def stats_params(l, want_agg):
                """Consume arbn_out holding layer-l stats.  Returns dict with
                scl [128,2] f32, nshift_fm [128,2] f32 (= -shift), and if
                want_agg: m_rep [128,256] bf16, negrow [1,256] bf16, w1s."""
                stg = spool.tile([128, 4], FP32, tag="stg")
                nc.sync.dma_start(stg[:], arbn_out[:])
                scl = ppool.tile([128, 2], FP32, tag="scl", name="scl")
                shift = ppool.tile([128, 2], FP32, tag="shift", name="shift")
                for m in range(2):
                    mean = spool.tile([128, 1], FP32, tag="bmean")
                    e2 = spool.tile([128, 1], FP32, tag="be2")
                    nc.vector.tensor_scalar(mean[:], stg[:, 2 * m:2 * m + 1],
                                            1.0 / C, None, op0=ALU.mult)
                    nc.vector.tensor_scalar(e2[:], stg[:, 2 * m + 1:2 * m + 2],
                                            1.0 / C, None, op0=ALU.mult)
                    var = spool.tile([128, 1], FP32, tag="bvar")
                    nc.vector.tensor_tensor(var[:], mean[:], mean[:], op=ALU.mult)
                    nc.vector.tensor_tensor(var[:], e2[:], var[:], op=ALU.subtract)
                    nc.vector.tensor_scalar(var[:], var[:], float(BN_EPS), None,
                                            op0=ALU.add)
                    sd = spool.tile([128, 1], FP32, tag="bsd")
                    nc.scalar.activation(sd[:], var[:], AF.Sqrt)
                    inv_s = spool.tile([128, 1], FP32, tag="binv")
                    nc.vector.tensor_tensor(inv_s[:], sd[:],
                                            gaminv_sb[:, 2 * l + m:2 * l + m + 1],
                                            op=ALU.mult)
                    nc.vector.reciprocal(scl[:, m:m + 1], inv_s[:])
                    tmp = spool.tile([128, 1], FP32, tag="btmp")
                    nc.vector.tensor_tensor(tmp[:], inv_s[:],
                                            bet_sb[:, 2 * l + m:2 * l + m + 1],
                                            op=ALU.mult)
                    nc.vector.tensor_tensor(shift[:, m:m + 1],
                                            mean[:], tmp[:], op=ALU.subtract)
                nshift_fm = ppool.tile([128, 2], FP32, tag="nshf", name="nshf")
                nc.vector.tensor_scalar(nshift_fm[:], shift[:], -1.0, None,
                                        op0=ALU.mult)
                prm = {"scl": scl, "nshift_fm": nshift_fm}
                if want_agg:
                    shift_bf = spool.tile([128, 2], BF16, tag="shbf")
                    nc.vector.tensor_copy(shift_bf[:], shift[:])
                    shrow = ppool.tile([1, 256], BF16, tag="shrow", name="shrow")
                    negrow = ppool.tile([1, 256], BF16, tag="negrow", name="negrow")
                    for m in range(2):
                        rps = ps_msc.tile([1, 128], FP32, tag="msc")
                        nc.tensor.matmul(rps[:], shift_bf[:, m:m + 1], ident[:],
                                         start=True, stop=True)
                        nc.vector.tensor_copy(shrow[:, 128 * m:128 * (m + 1)], rps[:])
                        nc.vector.tensor_scalar(negrow[:, 128 * m:128 * (m + 1)],
                                                rps[:], -1.0, None, op0=ALU.mult)
                    mps = ps_msc.tile([128, 256], FP32, tag="msc")
                    nc.tensor.matmul(mps[:], ones_col[:], shrow[:],
                                     start=True, stop=True)
                    m_rep = ppool.tile([128, 256], BF16, tag="mrep", name="mrep")
                    nc.vector.tensor_copy(m_rep[:], mps[:])
                    w1s = ppool.tile([128, 512], FP32R, tag="w1s", name="w1s")
                    lw = l + 1
                    for k in range(2):
                        for m in range(2):
                            col = ((lw * 2 + k) * 2 + m) * 128
                            nc.scalar.activation(
                                w1s[:, (2 * k + m) * 128:(2 * k + m + 1) * 128],
                                w1_sb[:, col:col + 128], AF.Identity,
                                scale=scl[:, k:k + 1])
                    prm.update(m_rep=m_rep, negrow=negrow, w1s=w1s)
                return prm

            # ---------- u_own = relu(z2 - shift), in place on z2 -----------
            # (z2's other consumers - stats, slice transposes - precede this)
            def compute_u(z2, nshift_fm):
                for m in range(2):
                    for jj in range(MJ):
                        lo = jj * 512
                        nw = min(512, NC - lo)
                        nc.scalar.activation(z2[m][:, lo:lo + nw],
                                             z2[m][:, lo:lo + nw], AF.Relu,
                                             bias=nshift_fm[:, m:m + 1])
                return z2

            # ---------- pooling of layer l (h = scl * u) -------------------
            def pooling(l, u, scl):
                pooled_ps = ps_msc.tile([128, 256], FP32, tag="pool", bufs=1)
                for j in range(NCH):
                    lo = j * 128
                    cw = min(128, NC - lo)
                    unm = tpool.tile([128, 256], BF16, tag="unm", bufs=1)
                    for m in range(2):
                        tp = ps_msc.tile([128, 128], BF16, tag="msc")
                        nc.tensor.transpose(tp[:cw, :], u[m][:, lo:lo + cw],
                                            ident[:])
                        nc.vector.tensor_copy(unm[:cw, 128 * m:128 * (m + 1)],
                                              tp[:cw, :])
                    nc.tensor.matmul(pooled_ps[:],
                                     pooh_sb[:cw, j * 128:(j + 1) * 128],
                                     unm[:cw, :],
                                     start=(j == 0), stop=(j == NCH - 1))
                pooled_sb = tpool.tile([128, 256], BF16, tag="pooled", bufs=1)
                nc.vector.tensor_copy(pooled_sb[:], pooled_ps[:])
                for m in range(2):
                    pl_ps = ps_msc.tile([128, 512], FP32, tag="msc")
                    nc.tensor.matmul(pl_ps[:], pooled_sb[:, 128 * m:128 * (m + 1)],
                                     placem_sb[:], start=True, stop=True)
                    gp = tpool.tile([128, 512], FP32, tag="gp", bufs=1)
                    nc.scalar.activation(gp[:], pl_ps[:], AF.Identity,
                                         scale=scl[:, m:m + 1])
                    nc.scalar.dma_start(arp_in[(l * 2 + m) * 128:(l * 2 + m + 1) * 128, :],
                                        gp[:])

            # ---------- raw-z2 transposes -> slice writes ------------------
            def slice_writes(z2):
                for j in range(NCH):
                    lo = j * 128
                    cw = min(128, NC - lo)
                    znm = tpool.tile([128, 256], BF16, tag="znm")
                    for m in range(2):
                        tp = ps_msc.tile([128, 128], BF16, tag="msc")
                        nc.tensor.transpose(tp[:cw, :], z2[m][:, lo:lo + cw],
                                            ident[:])
                        nc.vector.tensor_copy(znm[:cw, 128 * m:128 * (m + 1)],
                                              tp[:cw, :])
                    for (a, b) in ((lo, min(lo + cw, HALF)), (max(lo, HALF), lo + cw)):
                        if b <= a:
                            continue
                        hh = 0 if a < HALF else 1
                        r0 = a - hh * HALF
                        nc.scalar.dma_start(sliceT[hh][r0:r0 + (b - a), :],
                                            znm[a - lo:b - lo, :])

            # ---------- L0 x stream: host-pregathered rows, plain DMA ------
            def emit_xg_loads(ph):
                tiles = []
                K = int(K_pass[ph])
                for i in range(n_ginst[ph]):
                    c0 = i * GI_CHUNKS
                    nch = min(GI_CHUNKS, K - c0)
                    g = gpool[ph].tile([128, GI_CHUNKS, 128], BF16, tag="gx",
                                       name=f"gx{ph}t")
                    eng = nc.sync if i % 2 == 0 else nc.scalar
                    eng.dma_start(g[:, :nch, :],
                                  xg_in[ph][:, c0 * 128:(c0 + nch) * 128])
                    tiles.append(g)
                return tiles

            # ================= LAYER 0: x-space agg + encoder ==============
            xTown_sb = zpool.tile([128, NC], BF16, tag="z2_0", name="xTown_sb")
            nc.sync.dma_start(xTown_sb[:], xTown[:])
            gt = [emit_xg_loads(0), emit_xg_loads(1)]
            getg = [GetG(0, gt[0], None), GetG(1, gt[1], None)]
            oh_maps = [OhLazy(0, int(K_pass[0])),
                       OhLazy(int(K_pass[0]), int(K_pass[1]))]
            xsum = zpool.tile([128, NC], BF16, tag="z2_1", name="xsum")
            # sequential passes: all pass-0 windows, then all pass-1 windows
            # (interleaving would put early pass-1 matmuls ahead of the late
            # pass-0 windows whose completion frees the pass-1 gather slots)
            for ph in range(2):
                sp = 0
                for w in range(W):
                    lo = w * 128
                    cw = min(128, NC - lo)
                    kf = int(k_fix[ph, w])
                    if kf == 0:
                        if ph == 0 and int(k_fix[1, w]) == 0:
                            nc.vector.tensor_copy(xsum[:, lo:lo + cw],
                                                  xTown_sb[:, lo:lo + cw])
                        continue
                    aggt = ps_agg.tile([128, 256], FP32, tag="agg", name="aggx")
                    agg = aggt[:, 0:128]
                    for j in range(kf):
                        ci = sp + j
                        gsl = getg[ph](ci)
                        oh, col0 = oh_maps[ph](ci)
                        nc.tensor.matmul(agg[:, :], gsl, oh[:, col0:col0 + 128],
                                         start=(j == 0), stop=(j == kf - 1))
                    if ph == 0 or int(k_fix[0, w]) == 0:
                        src2 = xTown_sb
                    else:
                        src2 = xsum
                    nc.vector.tensor_tensor(xsum[:, lo:lo + cw], agg[:, :cw],
                                            src2[:, lo:lo + cw], op=ALU.add)
                    sp += kf

            # encoder on aggregated x: z_pre0 = xsum @ W_enc (+ bias terms)
            z_pre = [zpool.tile([128, NC], FP32R, tag=f"zpre{m}", name=f"zpre{m}")
                     for m in range(2)]
            for jj in range(MJ):
                lo = jj * 512
                nw = min(512, NC - lo)
                for m in range(2):
                    ps = ps_mlp.tile([128, 512], FP32, tag="mlp")
                    last = not p.benc_nonzero
                    nc.tensor.matmul(ps[:, :nw],
                                     wenc_sb[:, 128 * m:128 * (m + 1)],
                                     xsum[:, lo:lo + nw], start=True, stop=last)
                    if p.benc_nonzero:
                        # + b_enc (x) deg: the deg part of (1+deg) b_enc; the
                        # +1 part comes via the activation bias below.
                        nc.tensor.matmul(ps[:, :nw],
                                         bencrow_sb[:, 128 * m:128 * (m + 1)],
                                         deg_sb[:, lo:lo + nw],
                                         start=False, stop=True)
                    nc.scalar.activation(z_pre[m][:, lo:lo + nw], ps[:, :nw],
                                         AF.Identity,
                                         bias=bencc_sb[:, m:m + 1])

            def dump(nm, tiles):
                if not dbg:
                    return
                if not isinstance(tiles, list):
                    tiles = [tiles]
                for m, t in enumerate(tiles):
                    tt = t[:]
                    if tt.dtype == FP32R:
                        tt = tt.bitcast(FP32)
                    nc.sync.dma_start(
                        dbg_t[nm][:, m * NC:(m + 1) * NC] if len(tiles) > 1
                        else dbg_t[nm][:, :], tt)

            dump("d_xsum", xsum)
            dump("d_zpre0", z_pre)
            z2 = [zpool.tile([128, NC], BF16, tag=f"z2_{m}", name=f"z2_{m}")
                  for m in range(2)]
            mlp(0, z_pre, z2, w1_sb, lambda k, m: ((0 * 2 + k) * 2 + m) * 128)
            dump("d_z20", z2)
            stats_ar(z2)
            slice_writes(z2)
            if not skip_coll:
                nc.gpsimd.collective_compute(
                    "AllGather", ALU.bypass, replica_groups=RG,
                    ins=[sliceT[0].opt()], outs=[tableL.opt()])
                nc.gpsimd.collective_compute(
                    "AllGather", ALU.bypass, replica_groups=RG,
                    ins=[sliceT[1].opt()], outs=[tableH.opt()])

            # ================= LAYERS 1..L-1 ===============================
            for l in range(1, L):
                prm = stats_params(l - 1, want_agg=True)
                u = compute_u(z2, prm["nshift_fm"])
                if l == 1:
                    dump("d_u0", u)
                    if dbg:
                        nc.sync.dma_start(dbg_t["d_mrep"][:, :],
                                          prm["m_rep"][:])
                pooling(l - 1, u, prm["scl"])

                gt = [emit_gathers(0, half_t[0], 256),
                      emit_gathers(1, half_t[1], 256)]
                getg = [GetG(0, gt[0], prm["m_rep"]), GetG(1, gt[1], prm["m_rep"])]
                oh_maps = [OhLazy(0, int(K_pass[0])),
                           OhLazy(int(K_pass[0]), int(K_pass[1]))]
                z_pre = [zpool.tile([128, NC], FP32R, tag=f"zpre{m}",
                                    name=f"zpre{m}_{l}") for m in range(2)]
                # pass 0: chunks + rank-1 (-shift x deg), then + u_own
                sp = 0
                for w in range(W):
                    lo = w * 128
                    cw = min(128, NC - lo)
                    kf = int(k_fix[0, w])
                    aggt = ps_agg.tile([128, 256], FP32, tag="agg", name="aggp0")
                    # PSUM accumulation chains must not interleave within a
                    # tile: run the m=0 chain to completion, then m=1.
                    for m in range(2):
                        for j in range(kf):
                            ci = sp + j
                            gsl = getg[0](ci)
                            oh, col0 = oh_maps[0](ci)
                            nc.tensor.matmul(
                                aggt[:, 128 * m:128 * (m + 1)],
                                gsl[:, 128 * m:128 * (m + 1)],
                                oh[:, col0:col0 + 128], start=(j == 0), stop=False)
                        nc.tensor.matmul(
                            aggt[:, 128 * m:128 * (m + 1)],
                            prm["negrow"][:, 128 * m:128 * (m + 1)],
                            deg_sb[:, lo:lo + 128], start=(kf == 0), stop=True)
                        nc.vector.tensor_tensor(z_pre[m][:, lo:lo + cw],
                                                aggt[:, 128 * m:128 * m + cw],
                                                u[m][:, lo:lo + cw], op=ALU.add)
                    sp += kf
                # pass 1: chunks accumulate on top
                sp = 0
                for w in range(W):
                    kf = int(k_fix[1, w])
                    if kf == 0:
                        continue
                    lo = w * 128
                    cw = min(128, NC - lo)
                    aggt = ps_agg.tile([128, 256], FP32, tag="agg", name="aggp1")
                    for m in range(2):
                        for j in range(kf):
                            ci = sp + j
                            gsl = getg[1](ci)
                            oh, col0 = oh_maps[1](ci)
                            nc.tensor.matmul(
                                aggt[:, 128 * m:128 * (m + 1)],
                                gsl[:, 128 * m:128 * (m + 1)],
                                oh[:, col0:col0 + 128],
                                start=(j == 0), stop=(j == kf - 1))
                        nc.vector.tensor_tensor(z_pre[m][:, lo:lo + cw],
                                                aggt[:, 128 * m:128 * m + cw],
                                                z_pre[m][:, lo:lo + cw], op=ALU.add)
                    sp += kf

                dump(f"d_zpre{l}", z_pre)
                z2 = [zpool.tile([128, NC], BF16, tag=f"z2_{m}",
                                 name=f"z2_{m}_{l}") for m in range(2)]
                mlp(l, z_pre, z2, prm["w1s"], lambda k, m: (2 * k + m) * 128)
                if l == 1:
                    dump("d_z21", z2)
                stats_ar(z2)
                if l < L - 1:
                    slice_writes(z2)
                    if not skip_coll:
                        nc.gpsimd.collective_compute(
                            "AllGather", ALU.bypass, replica_groups=RG,
                            ins=[sliceT[0].opt()], outs=[tableL.opt()])
                        nc.gpsimd.collective_compute(
                            "AllGather", ALU.bypass, replica_groups=RG,
                            ins=[sliceT[1].opt()], outs=[tableH.opt()])

            # ================= final layer stats + pool + head =============
            prm = stats_params(L - 1, want_agg=False)
            u = compute_u(z2, prm["nshift_fm"])
            pooling(L - 1, u, prm["scl"])

            if not skip_coll:
                nc.gpsimd.collective_compute(
                    "AllReduce", ALU.add, replica_groups=RG,
                    ins=[arp_in.opt()], outs=[arp_out.opt()])
            y1ps = ps_mlp.tile([128, 512], FP32, tag="mlp")
            gtiles = []
            for k in range(DL // 128):
                gk = tpool.tile([128, 512], FP32, tag="gark", bufs=1)
                eng = nc.sync if k % 2 == 0 else nc.scalar
                eng.dma_start(gk[:], arp_out[128 * k:128 * (k + 1), :])
                gtiles.append(gk)
            for k in range(DL // 128):
                nc.tensor.matmul(y1ps[:], wfc1_sb[:, 128 * k:128 * (k + 1)],
                                 gtiles[k][:], start=(k == 0), stop=(k == DL // 128 - 1))
            y1 = tpool.tile([128, 512], FP32, tag="y1", bufs=1)
            nc.scalar.activation(y1[:], y1ps[:], AF.Relu, bias=bfc1_sb[:])
            y2ps = ps_msc.tile([1, 512], FP32, tag="msc")
            nc.tensor.matmul(y2ps[:], wfc2_sb[:], y1[:], start=True, stop=True)
            osb = tpool.tile([1, 512], FP32, tag="osb")
            nc.scalar.activation(osb[:], y2ps[:], AF.Identity, bias=bfc2_sb[:])
            nc.sync.dma_start(out_d[:], osb[:])

        if reps:
            with tc.For_i(0, reps, 1):
                emit_body()
        else:
            emit_body()

    nc.compile()
    return nc


# ==================================================================== run
_CACHE = {}


def _get_runner(p):
    import jax
    from jax.sharding import Mesh, PartitionSpec
    from jax.experimental.shard_map import shard_map
    from concourse.bass2jax import _bass_exec_p, install_neuronx_cc_hook

    nc = build_program(p)
    install_neuronx_cc_hook()
    part_name = nc.partition_id_tensor.name if nc.partition_id_tensor else None
    in_names, out_names, out_avals, zero_outs = [], [], [], []
    for alloc in nc.m.functions[0].allocations:
        if not isinstance(alloc, mybir.MemoryLocationSet):
            continue
        name = alloc.memorylocations[0].name
        if alloc.kind == "ExternalInput":
            if name != part_name:
                in_names.append(name)
        elif alloc.kind == "ExternalOutput":
            out_names.append(name)
            shape = tuple(alloc.tensor_shape)
            dtype = mybir.dt.np(alloc.dtype)
            out_avals.append(jax.core.ShapedArray(shape, dtype))
            zero_outs.append(np.zeros(shape, dtype))
    n_params = len(in_names)
    all_in_names = list(in_names) + list(out_names)
    if part_name is not None:
        all_in_names.append(part_name)

    def _body(*args):
        from concourse.bass2jax import partition_id_tensor
        operands = list(args)
        if part_name is not None:
            operands.append(partition_id_tensor())
        outs = _bass_exec_p.bind(
            *operands, out_avals=tuple(out_avals), in_names=tuple(all_in_names),
            out_names=tuple(out_names), lowering_input_output_aliases=(),
            sim_require_finite=False, sim_require_nnan=False, nc=nc)
        return tuple(outs)

    devices = jax.devices()[:N_CORES]
    mesh = Mesh(np.asarray(devices), ("core",))
    specs = (PartitionSpec("core"),) * (n_params + len(out_names))
    fn = jax.jit(shard_map(_body, mesh=mesh, in_specs=specs,
                           out_specs=(PartitionSpec("core"),) * len(out_names),
                           check_rep=False), keep_unused=True)
    return nc, fn, in_names, out_names, out_avals, zero_outs, mesh


def _device_args(p):
    import jax
    from jax.sharding import NamedSharding, PartitionSpec
    nc, fn, in_names, out_names, out_avals, zero_outs, mesh = _CACHE["runner"]
    per_core_maps = []
    for c in range(N_CORES):
        m = dict(p.shared)
        m.update(p.per_core[c])
        per_core_maps.append(m)
    concat_in = [np.concatenate([np.asarray(per_core_maps[c][nm])[None]
                                 for c in range(N_CORES)], axis=0)
                 .reshape(-1, *np.asarray(per_core_maps[0][nm]).shape[1:])
                 for nm in in_names]
    concat_zero = [np.zeros((N_CORES * z.shape[0], *z.shape[1:]), z.dtype)
                   for z in zero_outs]
    sh = NamedSharding(mesh, PartitionSpec("core"))
    args = [jax.device_put(a, sh) for a in concat_in + concat_zero]
    for a in args:
        a.block_until_ready()
    return args


def run_on_device(p):
    import jax
    sig = (p.N, p.E, p.G, p.Ktot, tuple(map(int, p.K_pass)),
           tuple(map(int, p.k_fix.ravel())))
    if _CACHE.get("sig") != sig:
        _CACHE.clear()
        _CACHE["sig"] = sig
    if "runner" not in _CACHE:
        _CACHE["runner"] = _get_runner(p)
    if "args" not in _CACHE:
        _CACHE["args"] = _device_args(p)
    nc, fn, in_names, out_names, out_avals, zero_outs, mesh = _CACHE["runner"]
    outs = fn(*_CACHE["args"])
    for o in outs:
        o.block_until_ready()
    res = np.asarray(outs[out_names.index("out")])
    res = res.reshape(N_CORES, 1, 512)[0, 0]     # core 0
    return res


def kernel(**inputs):
    p = preprocess(**inputs)
    _CACHE.pop("args", None)       # force fresh input upload for new data
    out = run_on_device(p)
    return out[:p.G].astype(np.float32).reshape(p.G, 1)



# revision 28
# speedup vs baseline: 2.2427x; 2.2427x over previous
"""Trainium2 Bass kernel for a 3-layer GIN-style GNN (nn_BaseGNN).

Sharding: data-parallel over nodes/edges by dst-owner across 8 NeuronCores.
Aggregation = one-hot matmuls over 128-edge chunks (PSUM accumulate), edge
source rows fetched from a replicated table in DRAM via gpsimd dma_gather.

Key structure (v2):
 - Layer 0 aggregates raw x (128-dim) and applies the encoder afterwards by
   linearity: z0 = (x_own + sum_j x_j) @ W_enc + (1+deg) b_enc.  The x table
   is a static replicated input; no initial table build or AllGather.
 - BatchNorm is algebraically folded: with s = gamma/sigma > 0,
   h = relu(s*(z - shift)) = s * relu(z - shift), so the inter-layer tables
   store RAW z2.  relu(z - shift) = max(z, shift) - shift; the max is applied
   per gathered tile, and the -shift*deg rank-1 term is injected into the
   aggregation PSUM via a 1-partition matmul with the in-degree vector.
   s is folded into the next layer's W1 (per-partition scale) and into the
   pooling output.  Hence slice writes (raw z2 transposes) do not wait for
   the BN stats AllReduce, which overlaps the table AllGathers.
 - Pooling of layer l runs inside layer l+1 (after global stats arrive),
   off the critical path.  The last layer keeps stats AllReduce + pool.

All instruction streams are identical across cores (SPMD); per-core
variation lives exclusively in input data.
"""

import numpy as np
import ml_dtypes

import concourse.bass as bass
import concourse.bacc as bacc
import concourse.mybir as mybir
import concourse.tile as tile
from concourse.masks import make_identity

BF16 = mybir.dt.bfloat16
FP32 = mybir.dt.float32
I16 = mybir.dt.int16
FP32R = mybir.dt.float32r
AF = mybir.ActivationFunctionType
ALU = mybir.AluOpType

N_CORES = 8
GI_CHUNKS = 6           # 128-edge chunks per dma_gather instruction
                        # (768 idxs = 768 descs leaves slack in the 1024-desc
                        # SWDGE queue ring; bigger gathers block on their drain)
CB = 8                 # one-hot chunks per batched is_equal
N_SWDGE_QUEUES = 4
BN_EPS = 1e-5


def _cdiv(a, b):
    return (a + b - 1) // b


class Plan:
    pass


# ==================================================================== host
def preprocess(x, edge_index, batch, num_graphs, W_enc, b_enc, W1, b1, W2, b2,
               gamma, beta, W_fc1, b_fc1, W_fc2, b_fc2):
    p = Plan()
    N, F_IN = x.shape
    D = W_enc.shape[1]
    L = W1.shape[0]
    G = int(num_graphs)
    E = edge_index.shape[1]
    C = N_CORES
    assert N % C == 0
    NC = N // C
    assert NC % 2 == 0
    HALF = NC // 2
    W = _cdiv(NC, 128)
    p.N, p.F_IN, p.D, p.L, p.G, p.E = N, F_IN, D, L, G, E
    p.NC, p.HALF, p.W = NC, HALF, W
    assert D == 256 and F_IN == 128, "layout hardcodes D=256, F_IN=128"
    assert HALF < 32768, "int16 gather index range"
    assert np.all(np.asarray(gamma) > 0), "BN fold requires gamma > 0"

    src = np.asarray(edge_index[0], np.int64)
    dst = np.asarray(edge_index[1], np.int64)
    batch = np.asarray(batch, np.int64)

    owner = dst // NC
    src_owner = src // NC
    src_local = src % NC
    src_half = (src_local >= HALF).astype(np.int64)
    table_row = HALF * src_owner + (src_local % HALF)
    dst_local = dst - owner * NC
    win = dst_local // 128

    counts = np.zeros((C, 2, W), np.int64)
    np.add.at(counts, (owner, src_half, win), 1)
    k_fix = _cdiv(counts, 128).max(axis=0)          # [2, W]
    p.k_fix = k_fix
    K_pass = k_fix.sum(axis=1).astype(np.int64)
    p.K_pass = K_pass
    p.Ktot = int(K_pass.sum())

    order = np.lexsort((dst_local, win, src_half, owner))
    so_owner = owner[order]
    so_half = src_half[order]
    so_win = win[order]
    so_row = table_row[order]
    so_dstloc = (dst_local - win * 128)[order]

    # chunk slot base per (p, w) in each pass stream
    slot_base = np.zeros((2, W), np.int64)
    for ph in range(2):
        b = 0
        for w in range(W):
            slot_base[ph, w] = b
            b += int(k_fix[ph, w]) * 128

    # per-(c,p,w) edge segment boundaries in the sorted arrays
    seg = np.zeros((C, 2, W, 2), np.int64)
    keys = ((so_owner * 2 + so_half) * W + so_win)
    bounds = np.searchsorted(keys, np.arange(C * 2 * W + 1))
    for c in range(C):
        for ph in range(2):
            for w in range(W):
                kk = (c * 2 + ph) * W + w
                seg[c, ph, w] = bounds[kk], bounds[kk + 1]

    def wrap16(lin):
        S = lin.shape[0] // 16
        t = lin.reshape(S, 16).T
        return np.ascontiguousarray(np.tile(t, (8, 1)).astype(np.int16))

    cnt = np.bincount(batch, minlength=G).astype(np.float64)
    cnt_inv = (1.0 / np.maximum(cnt, 1.0)).astype(np.float32)

    WPAD = W * 128
    xf = np.asarray(x, np.float32)
    xb_full = xf.astype(ml_dtypes.bfloat16).reshape(C, 2, HALF, F_IN)
    xtabs_bf = [np.ascontiguousarray(xb_full[:, 0].reshape(C * HALF, F_IN)),
                np.ascontiguousarray(xb_full[:, 1].reshape(C * HALF, F_IN))]
    dst_iota = np.arange(128, dtype=np.float32)
    p.per_core = []
    for c in range(C):
        d = {}
        for ph in range(2):
            K = int(K_pass[ph])
            idx = np.zeros((K * 128,), np.int16)
            dl = np.full((K * 128,), -1.0, np.float32)
            for w in range(W):
                a, b = seg[c, ph, w]
                n = b - a
                sb = int(slot_base[ph, w])
                idx[sb:sb + n] = so_row[a:b].astype(np.int16)
                dl[sb:sb + n] = so_dstloc[a:b].astype(np.float32)
            d[f"idx{ph}"] = wrap16(idx)
            # host-pregathered x rows for layer 0: [128 slot, K, 128 feat]
            xg = np.take(xtabs_bf[ph], idx.reshape(K, 128).astype(np.int64),
                         axis=0)                      # [K, 128, F_IN]
            d[f"xg{ph}"] = np.ascontiguousarray(
                xg.transpose(1, 0, 2).reshape(128, K * F_IN))
            if ph == 0:
                dl0 = dl
            else:
                dl = np.concatenate([dl0, dl])
        d["dstloc"] = np.ascontiguousarray(
            dl.reshape(p.Ktot, 128).T).astype(ml_dtypes.bfloat16)
        nb = batch[c * NC:(c + 1) * NC]
        g_lo = int(nb[0])
        span = int(nb[-1]) - g_lo + 1
        assert span <= 128, f"core {c} spans {span} graphs"
        bl = np.full((W * 128,), -1.0, np.float32)
        bl[:NC] = (nb - g_lo).astype(np.float32)
        blm = bl.reshape(W, 128).T                    # [128 node, W]
        # pooling one-hot const: pooh[p, j, d] = (batchloc[p,j] == d)
        d["pooh"] = np.ascontiguousarray(
            (blm[:, :, None] == dst_iota[None, None, :])
            .astype(ml_dtypes.bfloat16).reshape(128, W * 128))
        pl = np.zeros((128, 512), np.float32)
        hi = min(128, G - g_lo)
        pl[np.arange(hi), g_lo + np.arange(hi)] = cnt_inv[g_lo:g_lo + hi]
        d["placem"] = pl.astype(ml_dtypes.bfloat16)
        d["xTown"] = np.ascontiguousarray(
            xf[c * NC:(c + 1) * NC].T).astype(ml_dtypes.bfloat16)
        degc = np.bincount(dst_local[owner == c], minlength=NC).astype(np.float32)
        dv = np.zeros((1, WPAD), np.float32)
        dv[0, :NC] = degc
        d["deg"] = dv.astype(ml_dtypes.bfloat16)
        p.per_core.append(d)

    sh = {}
    sh["iota"] = np.tile(np.arange(128, dtype=np.float32),
                         (128, CB)).astype(ml_dtypes.bfloat16)
    sh["wenc"] = np.asarray(W_enc, np.float32).astype(ml_dtypes.bfloat16)
    w1c = np.zeros((128, L * 4 * 128), np.float32)
    w2c = np.zeros((128, L * 4 * 128), np.float32)
    for l in range(L):
        for k in range(2):
            for m in range(2):
                col = ((l * 2 + k) * 2 + m) * 128
                w1c[:, col:col + 128] = W1[l, 128 * k:128 * (k + 1), 128 * m:128 * (m + 1)]
                w2c[:, col:col + 128] = W2[l, 128 * k:128 * (k + 1), 128 * m:128 * (m + 1)]
    sh["w1"] = w1c
    sh["w2"] = w2c

    def col2(v):
        return np.ascontiguousarray(np.asarray(v, np.float32).reshape(2, 128).T)

    sh["bencc"] = col2(b_enc)
    sh["bencrow"] = np.asarray(b_enc, np.float32).reshape(1, 256).astype(
        ml_dtypes.bfloat16)
    p.benc_nonzero = bool(np.any(np.asarray(b_enc) != 0))
    sh["b1"] = np.concatenate([col2(b1[l]) for l in range(L)], axis=1)
    sh["b2"] = np.concatenate([col2(b2[l]) for l in range(L)], axis=1)
    sh["gaminv"] = np.concatenate(
        [col2(1.0 / np.asarray(gamma[l], np.float64)) for l in range(L)], axis=1)
    sh["bet"] = np.concatenate([col2(beta[l]) for l in range(L)], axis=1)
    DL = D * L
    sh["wfc1"] = np.ascontiguousarray(
        np.asarray(W_fc1, np.float32).reshape(DL // 128, 128, 128)
        .transpose(1, 0, 2).reshape(128, DL))
    sh["wfc2"] = np.asarray(W_fc2, np.float32).reshape(128, 1)
    sh["bfc1"] = np.asarray(b_fc1, np.float32).reshape(128, 1)
    sh["bfc2"] = np.asarray(b_fc2, np.float32).reshape(1, 1)
    p.shared = sh
    return p


# ================================================================= program
def build_program(p, reps=0, skip_coll=False, dbg=False):
    C = N_CORES
    N, D, L, W, NC, HALF = p.N, p.D, p.L, p.W, p.NC, p.HALF
    NROW = C * HALF
    K_pass, k_fix, Ktot = p.K_pass, p.k_fix, p.Ktot
    NCH = _cdiv(NC, 128)
    MJ = _cdiv(NC, 512)
    DL = D * L
    WPAD = W * 128

    nc = bacc.Bacc("TRN2", target_bir_lowering=False, debug=False,
                   enable_asserts=False, num_devices=C,
                   num_swdge_queues=N_SWDGE_QUEUES)

    ein = {}

    def EIN(name, shape, dt):
        ein[name] = nc.dram_tensor(name, list(shape), dt, kind="ExternalInput").ap()
        return ein[name]

    xTown = EIN("xTown", [128, NC], BF16)
    idx_in = [EIN("idx0", [128, int(K_pass[0]) * 8], I16),
              EIN("idx1", [128, int(K_pass[1]) * 8], I16)]
    xg_in = [EIN("xg0", [128, int(K_pass[0]) * 128], BF16),
             EIN("xg1", [128, int(K_pass[1]) * 128], BF16)]
    dstloc_in = EIN("dstloc", [128, Ktot], BF16)
    iota_in = EIN("iota", [128, CB * 128], BF16)
    pooh_in = EIN("pooh", [128, W * 128], BF16)
    placem_in = EIN("placem", [128, 512], BF16)
    deg_in = EIN("deg", [1, WPAD], BF16)
    wenc_in = EIN("wenc", [128, 256], BF16)
    w1_in = EIN("w1", [128, L * 4 * 128], FP32R)
    w2_in = EIN("w2", [128, L * 4 * 128], FP32R)
    bencc_in = EIN("bencc", [128, 2], FP32)
    bencrow_in = EIN("bencrow", [1, 256], BF16)
    b1_in = EIN("b1", [128, L * 2], FP32)
    b2_in = EIN("b2", [128, L * 2], FP32)
    gaminv_in = EIN("gaminv", [128, L * 2], FP32)
    bet_in = EIN("bet", [128, L * 2], FP32)
    wfc1_in = EIN("wfc1", [128, DL], FP32)
    wfc2_in = EIN("wfc2", [128, 1], FP32)
    bfc1_in = EIN("bfc1", [128, 1], FP32)
    bfc2_in = EIN("bfc2", [1, 1], FP32)

    out_d = nc.dram_tensor("out", [1, 512], FP32, kind="ExternalOutput").ap()
    dbg_t = {}
    if dbg:
        for nm, shape, dt_ in [
                ("d_xsum", [128, NC], BF16), ("d_zpre0", [128, 2 * NC], FP32),
                ("d_z20", [128, 2 * NC], BF16), ("d_u0", [128, 2 * NC], BF16),
                ("d_mrep", [128, 256], BF16), ("d_zpre1", [128, 2 * NC], FP32),
                ("d_z21", [128, 2 * NC], BF16), ("d_zpre2", [128, 2 * NC], FP32)]:
            dbg_t[nm] = nc.dram_tensor(nm, shape, dt_,
                                       kind="ExternalOutput").ap()

    tableL = nc.dram_tensor("tableL", [NROW, 256], BF16, kind="Internal").ap()
    tableH = nc.dram_tensor("tableH", [NROW, 256], BF16, kind="Internal").ap()
    sliceT = [nc.dram_tensor(f"slice{h}", [HALF, 256], BF16, kind="Internal").ap()
              for h in range(2)]
    arbn_in = nc.dram_tensor("arbn_in", [128, 4], FP32, kind="Internal").ap()
    arbn_out = nc.dram_tensor("arbn_out", [128, 4], FP32, kind="Internal",
                              addr_space="Shared").ap()
    arp_in = nc.dram_tensor("arp_in", [DL, 512], FP32, kind="Internal").ap()
    arp_out = nc.dram_tensor("arp_out", [DL, 512], FP32, kind="Internal",
                             addr_space="Shared").ap()

    RG = [list(range(C))]
    half_t = [tableL, tableH]

    import contextlib
    with tile.TileContext(nc) as tc, contextlib.ExitStack() as ctx:
        consts = ctx.enter_context(tc.tile_pool(name="consts", bufs=1))
        gpool_s = ctx.enter_context(tc.tile_pool(name="gP", bufs=10))
        gpool = [gpool_s, gpool_s]
        ohpool = ctx.enter_context(tc.tile_pool(name="oh", bufs=4))
        zpool = ctx.enter_context(tc.tile_pool(name="z", bufs=1))
        spool = ctx.enter_context(tc.tile_pool(name="s", bufs=2))
        tpool = ctx.enter_context(tc.tile_pool(name="t", bufs=2))
        ppool = ctx.enter_context(tc.tile_pool(name="prm", bufs=1))
        ps_agg = ctx.enter_context(tc.tile_pool(name="ps_agg", bufs=3, space="PSUM"))
        ps_mlp = ctx.enter_context(tc.tile_pool(name="ps_mlp", bufs=2, space="PSUM"))
        ps_msc = ctx.enter_context(tc.tile_pool(name="ps_msc", bufs=2, space="PSUM"))

        def load_const(apin, shape, dt):
            t = consts.tile(shape, dt, name="c_" + apin.tensor.name)
            nc.sync.dma_start(t[:], apin[:])
            return t

        pooh_sb = load_const(pooh_in, [128, W * 128], BF16)
        iota_sb = load_const(iota_in, [128, CB * 128], BF16)
        wenc_sb = load_const(wenc_in, [128, 256], BF16)
        w1_sb = load_const(w1_in, [128, L * 4 * 128], FP32R)
        w2_sb = load_const(w2_in, [128, L * 4 * 128], FP32R)
        bencc_sb = load_const(bencc_in, [128, 2], FP32)
        bencrow_sb = load_const(bencrow_in, [1, 256], BF16)
        b1_sb = load_const(b1_in, [128, L * 2], FP32)
        b2_sb = load_const(b2_in, [128, L * 2], FP32)
        gaminv_sb = load_const(gaminv_in, [128, L * 2], FP32)
        bet_sb = load_const(bet_in, [128, L * 2], FP32)
        wfc1_sb = load_const(wfc1_in, [128, DL], FP32)
        wfc2_sb = load_const(wfc2_in, [128, 1], FP32)
        bfc1_sb = load_const(bfc1_in, [128, 1], FP32)
        bfc2_sb = load_const(bfc2_in, [1, 1], FP32)
        dstloc_sb = load_const(dstloc_in, [128, Ktot], BF16)
        placem_sb = load_const(placem_in, [128, 512], BF16)
        deg_sb = load_const(deg_in, [1, WPAD], BF16)
        idx_sb = [load_const(idx_in[0], [128, int(K_pass[0]) * 8], I16),
                  load_const(idx_in[1], [128, int(K_pass[1]) * 8], I16)]
        ident = consts.tile([128, 128], BF16)
        make_identity(nc, ident[:])
        ones_col = consts.tile([1, 128], BF16)
        nc.vector.memset(ones_col[:], 1.0)

        n_ginst = [int(_cdiv(int(K_pass[ph]), GI_CHUNKS)) for ph in range(2)]

        def emit_body():
            # ---------- gathers helper ------------------------------------
            def emit_gathers(ph, table_ap, es):
                tiles = []
                K = int(K_pass[ph])
                for i in range(n_ginst[ph]):
                    c0 = i * GI_CHUNKS
                    nch = min(GI_CHUNKS, K - c0)
                    g = gpool[ph].tile([128, GI_CHUNKS, es], BF16, tag="g",
                                       name=f"g{ph}t")
                    nc.gpsimd.dma_gather(
                        g[:, :nch, :], table_ap[:], idx_sb[ph][:, c0 * 8:(c0 + nch) * 8],
                        num_idxs=nch * 128, num_idxs_reg=nch * 128, elem_size=es,
                        single_packet=False, queue_num=(i % N_SWDGE_QUEUES),
                    )
                    tiles.append(g)
                return tiles

            class GetG:
                """Lazily applies max(g, shift) (u-space relu prep) the first
                time a gather tile is touched, in consumption order, so the
                DVE stream order matches the window loops (no queue cycle)."""
                def __init__(self, ph, tiles, mrep):
                    self.ph, self.tiles, self.mrep = ph, tiles, mrep
                    self.maxed = set()

                def __call__(self, ci):
                    gi = ci // GI_CHUNKS
                    g = self.tiles[gi]
                    if self.mrep is not None and gi not in self.maxed:
                        nch = min(GI_CHUNKS, int(K_pass[self.ph]) - gi * GI_CHUNKS)
                        nc.vector.tensor_tensor(
                            g[:, :nch, :], g[:, :nch, :],
                            self.mrep[:].unsqueeze(1).to_broadcast([128, nch, 256]),
                            op=ALU.max)
                        self.maxed.add(gi)
                    return g[:, ci % GI_CHUNKS, :]

            class OhLazy:
                """Loads precomputed one-hot groups on first touch, in
                consumption order.  base = ohs column of this pass's chunk 0."""
                def __init__(self, base, K):
                    self.base, self.K = base, K
                    self.map = {}

                def __call__(self, ci):
                    if ci not in self.map:
                        g0 = ci - ci % CB
                        cn = min(CB, self.K - g0)
                        oh = ohpool.tile([128, CB * 128], BF16, tag="oh",
                                         name="oht")
                        c0 = self.base + g0
                        nc.vector.tensor_tensor(
                            oh[:, :cn * 128], iota_sb[:, :cn * 128],
                            dstloc_sb[:, c0:c0 + cn].to_broadcast([128, cn, 128]),
                            op=ALU.is_equal)
                        for k in range(cn):
                            self.map[g0 + k] = (oh, k * 128)
                    return self.map[ci]

            # ---------- shared MLP (z_pre -> z2), feature-major ------------
            def mlp(l, z_pre, z2, w1src, w1cols):
                for jj in range(MJ):
                    lo = jj * 512
                    nw = min(512, NC - lo)
                    z1t = [tpool.tile([128, 512], FP32R, tag=f"z1_{m}", bufs=1,
                                      name=f"z1t{m}") for m in range(2)]
                    for m in range(2):
                        ps = ps_mlp.tile([128, 512], FP32, tag="mlp")
                        for k in range(2):
                            col = w1cols(k, m)
                            nc.tensor.matmul(ps[:, :nw],
                                             w1src[:, col:col + 128],
                                             z_pre[k][:, lo:lo + nw],
                                             start=(k == 0), stop=(k == 1))
                        nc.scalar.activation(z1t[m][:, :nw], ps[:, :nw], AF.Relu,
                                             bias=b1_sb[:, 2 * l + m:2 * l + m + 1])
                    for m in range(2):
                        ps = ps_mlp.tile([128, 512], FP32, tag="mlp")
                        for k in range(2):
                            col = ((l * 2 + k) * 2 + m) * 128
                            nc.tensor.matmul(ps[:, :nw],
                                             w2_sb[:, col:col + 128],
                                             z1t[k][:, :nw],
                                             start=(k == 0), stop=(k == 1))
                        nc.scalar.activation(z2[m][:, lo:lo + nw], ps[:, :nw],
                                             AF.Identity,
                                             bias=b2_sb[:, 2 * l + m:2 * l + m + 1])

            # ---------- BN stats of z2 -> small AllReduce ------------------
            def stats_ar(z2):
                nbch = _cdiv(NC, 512)
                stt = spool.tile([128, 4], FP32, tag="stt")
                bnacc = tpool.tile([128, nbch, 6], FP32, tag="bnacc", bufs=1)
                for m in range(2):
                    for jj in range(nbch):
                        lo = jj * 512
                        nw = min(512, NC - lo)
                        nc.vector.bn_stats(bnacc[:, jj, :], z2[m][:, lo:lo + nw])
                    ag = spool.tile([128, 2], FP32, tag="bnag")
                    nc.vector.bn_aggr(ag[:], bnacc[:])
                    sq = spool.tile([128, 1], FP32, tag="bnsq")
                    nc.vector.tensor_tensor(sq[:], ag[:, 0:1], ag[:, 0:1],
                                            op=ALU.mult)
                    nc.vector.tensor_copy(stt[:, 2 * m:2 * m + 1], ag[:, 0:1])
                    nc.vector.tensor_tensor(stt[:, 2 * m + 1:2 * m + 2], ag[:, 1:2],
                                            sq[:], op=ALU.add)
                nc.sync.dma_start(arbn_in[:], stt[:])
                if not skip_coll:
                    nc.gpsimd.collective_compute(
                        "AllReduce", ALU.add, replica_groups=RG,
                        ins=[arbn_in.opt()], outs=[arbn_out.opt()])

            # ---------- stats -> scl / shift / rows / m_rep / w1s ----------
            def stats_params(l, want_agg):
                """Consume arbn_out holding layer-l stats.  Returns dict with
                scl [128,2] f32, nshift_fm [128,2] f32 (= -shift), and if
                want_agg: m_rep [128,256] bf16, negrow [1,256] bf16, w1s."""
                stg = spool.tile([128, 4], FP32, tag="stg")
                nc.sync.dma_start(stg[:], arbn_out[:])
                scl = ppool.tile([128, 2], FP32, tag="scl", name="scl")
                shift = ppool.tile([128, 2], FP32, tag="shift", name="shift")
                for m in range(2):
                    mean = spool.tile([128, 1], FP32, tag="bmean")
                    e2 = spool.tile([128, 1], FP32, tag="be2")
                    nc.vector.tensor_scalar(mean[:], stg[:, 2 * m:2 * m + 1],
                                            1.0 / C, None, op0=ALU.mult)
                    nc.vector.tensor_scalar(e2[:], stg[:, 2 * m + 1:2 * m + 2],
                                            1.0 / C, None, op0=ALU.mult)
                    var = spool.tile([128, 1], FP32, tag="bvar")
                    nc.vector.tensor_tensor(var[:], mean[:], mean[:], op=ALU.mult)
                    nc.vector.tensor_tensor(var[:], e2[:], var[:], op=ALU.subtract)
                    nc.vector.tensor_scalar(var[:], var[:], float(BN_EPS), None,
                                            op0=ALU.add)
                    sd = spool.tile([128, 1], FP32, tag="bsd")
                    nc.scalar.activation(sd[:], var[:], AF.Sqrt)
                    inv_s = spool.tile([128, 1], FP32, tag="binv")
                    nc.vector.tensor_tensor(inv_s[:], sd[:],
                                            gaminv_sb[:, 2 * l + m:2 * l + m + 1],
                                            op=ALU.mult)
                    nc.vector.reciprocal(scl[:, m:m + 1], inv_s[:])
                    tmp = spool.tile([128, 1], FP32, tag="btmp")
                    nc.vector.tensor_tensor(tmp[:], inv_s[:],
                                            bet_sb[:, 2 * l + m:2 * l + m + 1],
                                            op=ALU.mult)
                    nc.vector.tensor_tensor(shift[:, m:m + 1],
                                            mean[:], tmp[:], op=ALU.subtract)
                nshift_fm = ppool.tile([128, 2], FP32, tag="nshf", name="nshf")
                nc.vector.tensor_scalar(nshift_fm[:], shift[:], -1.0, None,
                                        op0=ALU.mult)
                prm = {"scl": scl, "nshift_fm": nshift_fm}
                if want_agg:
                    shift_bf = spool.tile([128, 2], BF16, tag="shbf")
                    nc.vector.tensor_copy(shift_bf[:], shift[:])
                    shrow = ppool.tile([1, 256], BF16, tag="shrow", name="shrow")
                    negrow = ppool.tile([1, 256], BF16, tag="negrow", name="negrow")
                    for m in range(2):
                        rps = ps_msc.tile([1, 128], FP32, tag="msc")
                        nc.tensor.matmul(rps[:], shift_bf[:, m:m + 1], ident[:],
                                         start=True, stop=True)
                        nc.vector.tensor_copy(shrow[:, 128 * m:128 * (m + 1)], rps[:])
                        nc.vector.tensor_scalar(negrow[:, 128 * m:128 * (m + 1)],
                                                rps[:], -1.0, None, op0=ALU.mult)
                    mps = ps_msc.tile([128, 256], FP32, tag="msc")
                    nc.tensor.matmul(mps[:], ones_col[:], shrow[:],
                                     start=True, stop=True)
                    m_rep = ppool.tile([128, 256], BF16, tag="mrep", name="mrep")
                    nc.vector.tensor_copy(m_rep[:], mps[:])
                    w1s = ppool.tile([128, 512], FP32R, tag="w1s", name="w1s")
                    lw = l + 1
                    for k in range(2):
                        for m in range(2):
                            col = ((lw * 2 + k) * 2 + m) * 128
                            nc.scalar.activation(
                                w1s[:, (2 * k + m) * 128:(2 * k + m + 1) * 128],
                                w1_sb[:, col:col + 128], AF.Identity,
                                scale=scl[:, k:k + 1])
                    prm.update(m_rep=m_rep, negrow=negrow, w1s=w1s)
                return prm

            # ---------- u_own = relu(z2 - shift), in place on z2 -----------
            # (z2's other consumers - stats, slice transposes - precede this)
            def compute_u(z2, nshift_fm):
                for m in range(2):
                    for jj in range(MJ):
                        lo = jj * 512
                        nw = min(512, NC - lo)
                        nc.scalar.activation(z2[m][:, lo:lo + nw],
                                             z2[m][:, lo:lo + nw], AF.Relu,
                                             bias=nshift_fm[:, m:m + 1])
                return z2

            # ---------- pooling of layer l (h = scl * u) -------------------
            def pooling(l, u, scl):
                pooled_ps = ps_msc.tile([128, 256], FP32, tag="pool", bufs=1)
                for j in range(NCH):
                    lo = j * 128
                    cw = min(128, NC - lo)
                    unm = tpool.tile([128, 256], BF16, tag="unm", bufs=1)
                    for m in range(2):
                        tp = ps_msc.tile([128, 128], BF16, tag="msc")
                        nc.tensor.transpose(tp[:cw, :], u[m][:, lo:lo + cw],
                                            ident[:])
                        nc.vector.tensor_copy(unm[:cw, 128 * m:128 * (m + 1)],
                                              tp[:cw, :])
                    nc.tensor.matmul(pooled_ps[:],
                                     pooh_sb[:cw, j * 128:(j + 1) * 128],
                                     unm[:cw, :],
                                     start=(j == 0), stop=(j == NCH - 1))
                pooled_sb = tpool.tile([128, 256], BF16, tag="pooled", bufs=1)
                nc.vector.tensor_copy(pooled_sb[:], pooled_ps[:])
                for m in range(2):
                    pl_ps = ps_msc.tile([128, 512], FP32, tag="msc")
                    nc.tensor.matmul(pl_ps[:], pooled_sb[:, 128 * m:128 * (m + 1)],
                                     placem_sb[:], start=True, stop=True)
                    gp = tpool.tile([128, 512], FP32, tag="gp", bufs=1)
                    nc.scalar.activation(gp[:], pl_ps[:], AF.Identity,
                                         scale=scl[:, m:m + 1])
                    nc.scalar.dma_start(arp_in[(l * 2 + m) * 128:(l * 2 + m + 1) * 128, :],
                                        gp[:])

            # ---------- raw-z2 transposes -> slice writes ------------------
            def slice_writes(z2):
                for j in range(NCH):
                    lo = j * 128
                    cw = min(128, NC - lo)
                    znm = tpool.tile([128, 256], BF16, tag="znm")
                    for m in range(2):
                        tp = ps_msc.tile([128, 128], BF16, tag="msc")
                        nc.tensor.transpose(tp[:cw, :], z2[m][:, lo:lo + cw],
                                            ident[:])
                        nc.vector.tensor_copy(znm[:cw, 128 * m:128 * (m + 1)],
                                              tp[:cw, :])
                    for (a, b) in ((lo, min(lo + cw, HALF)), (max(lo, HALF), lo + cw)):
                        if b <= a:
                            continue
                        hh = 0 if a < HALF else 1
                        r0 = a - hh * HALF
                        nc.scalar.dma_start(sliceT[hh][r0:r0 + (b - a), :],
                                            znm[a - lo:b - lo, :])

            # ---------- L0 x stream: host-pregathered rows, plain DMA ------
            def emit_xg_loads(ph):
                tiles = []
                K = int(K_pass[ph])
                for i in range(n_ginst[ph]):
                    c0 = i * GI_CHUNKS
                    nch = min(GI_CHUNKS, K - c0)
                    g = gpool[ph].tile([128, GI_CHUNKS, 128], BF16, tag="gx",
                                       bufs=4, name=f"gx{ph}t")
                    eng = nc.sync if i % 2 == 0 else nc.scalar
                    eng.dma_start(g[:, :nch, :],
                                  xg_in[ph][:, c0 * 128:(c0 + nch) * 128])
                    tiles.append(g)
                return tiles

            # ================= LAYER 0: x-space agg + encoder ==============
            xTown_sb = zpool.tile([128, NC], BF16, tag="z2_0", name="xTown_sb")
            nc.sync.dma_start(xTown_sb[:], xTown[:])
            gt = [emit_xg_loads(0), emit_xg_loads(1)]
            getg = [GetG(0, gt[0], None), GetG(1, gt[1], None)]
            oh_maps = [OhLazy(0, int(K_pass[0])),
                       OhLazy(int(K_pass[0]), int(K_pass[1]))]
            xsum = zpool.tile([128, NC], BF16, tag="z2_1", name="xsum")
            # sequential passes: all pass-0 windows, then all pass-1 windows
            # (interleaving would put early pass-1 matmuls ahead of the late
            # pass-0 windows whose completion frees the pass-1 gather slots)
            for ph in range(2):
                sp = 0
                for w in range(W):
                    lo = w * 128
                    cw = min(128, NC - lo)
                    kf = int(k_fix[ph, w])
                    if kf == 0:
                        if ph == 0 and int(k_fix[1, w]) == 0:
                            nc.vector.tensor_copy(xsum[:, lo:lo + cw],
                                                  xTown_sb[:, lo:lo + cw])
                        continue
                    aggt = ps_agg.tile([128, 256], FP32, tag="agg", name="aggx")
                    agg = aggt[:, 0:128]
                    for j in range(kf):
                        ci = sp + j
                        gsl = getg[ph](ci)
                        oh, col0 = oh_maps[ph](ci)
                        nc.tensor.matmul(agg[:, :], gsl, oh[:, col0:col0 + 128],
                                         start=(j == 0), stop=(j == kf - 1))
                    if ph == 0 or int(k_fix[0, w]) == 0:
                        src2 = xTown_sb
                    else:
                        src2 = xsum
                    nc.vector.tensor_tensor(xsum[:, lo:lo + cw], agg[:, :cw],
                                            src2[:, lo:lo + cw], op=ALU.add)
                    sp += kf

            # encoder on aggregated x: z_pre0 = xsum @ W_enc (+ bias terms)
            z_pre = [zpool.tile([128, NC], FP32R, tag=f"zpre{m}", name=f"zpre{m}")
                     for m in range(2)]
            for jj in range(MJ):
                lo = jj * 512
                nw = min(512, NC - lo)
                for m in range(2):
                    ps = ps_mlp.tile([128, 512], FP32, tag="mlp")
                    last = not p.benc_nonzero
                    nc.tensor.matmul(ps[:, :nw],
                                     wenc_sb[:, 128 * m:128 * (m + 1)],
                                     xsum[:, lo:lo + nw], start=True, stop=last)
                    if p.benc_nonzero:
                        # + b_enc (x) deg: the deg part of (1+deg) b_enc; the
                        # +1 part comes via the activation bias below.
                        nc.tensor.matmul(ps[:, :nw],
                                         bencrow_sb[:, 128 * m:128 * (m + 1)],
                                         deg_sb[:, lo:lo + nw],
                                         start=False, stop=True)
                    nc.scalar.activation(z_pre[m][:, lo:lo + nw], ps[:, :nw],
                                         AF.Identity,
                                         bias=bencc_sb[:, m:m + 1])

            def dump(nm, tiles):
                if not dbg:
                    return
                if not isinstance(tiles, list):
                    tiles = [tiles]
                for m, t in enumerate(tiles):
                    tt = t[:]
                    if tt.dtype == FP32R:
                        tt = tt.bitcast(FP32)
                    nc.sync.dma_start(
                        dbg_t[nm][:, m * NC:(m + 1) * NC] if len(tiles) > 1
                        else dbg_t[nm][:, :], tt)

            dump("d_xsum", xsum)
            dump("d_zpre0", z_pre)
            z2 = [zpool.tile([128, NC], BF16, tag=f"z2_{m}", name=f"z2_{m}")
                  for m in range(2)]
            mlp(0, z_pre, z2, w1_sb, lambda k, m: ((0 * 2 + k) * 2 + m) * 128)
            dump("d_z20", z2)
            stats_ar(z2)
            slice_writes(z2)
            if not skip_coll:
                nc.gpsimd.collective_compute(
                    "AllGather", ALU.bypass, replica_groups=RG,
                    ins=[sliceT[0].opt()], outs=[tableL.opt()])
                nc.gpsimd.collective_compute(
                    "AllGather", ALU.bypass, replica_groups=RG,
                    ins=[sliceT[1].opt()], outs=[tableH.opt()])

            # ================= LAYERS 1..L-1 ===============================
            for l in range(1, L):
                prm = stats_params(l - 1, want_agg=True)
                u = compute_u(z2, prm["nshift_fm"])
                if l == 1:
                    dump("d_u0", u)
                    if dbg:
                        nc.sync.dma_start(dbg_t["d_mrep"][:, :],
                                          prm["m_rep"][:])
                pooling(l - 1, u, prm["scl"])

                gt = [emit_gathers(0, half_t[0], 256),
                      emit_gathers(1, half_t[1], 256)]
                getg = [GetG(0, gt[0], prm["m_rep"]), GetG(1, gt[1], prm["m_rep"])]
                oh_maps = [OhLazy(0, int(K_pass[0])),
                           OhLazy(int(K_pass[0]), int(K_pass[1]))]
                z_pre = [zpool.tile([128, NC], FP32R, tag=f"zpre{m}",
                                    name=f"zpre{m}_{l}") for m in range(2)]
                # pass 0: chunks + rank-1 (-shift x deg), then + u_own
                sp = 0
                for w in range(W):
                    lo = w * 128
                    cw = min(128, NC - lo)
                    kf = int(k_fix[0, w])
                    aggt = ps_agg.tile([128, 256], FP32, tag="agg", name="aggp0")
                    # PSUM accumulation chains must not interleave within a
                    # tile: run the m=0 chain to completion, then m=1.
                    for m in range(2):
                        for j in range(kf):
                            ci = sp + j
                            gsl = getg[0](ci)
                            oh, col0 = oh_maps[0](ci)
                            nc.tensor.matmul(
                                aggt[:, 128 * m:128 * (m + 1)],
                                gsl[:, 128 * m:128 * (m + 1)],
                                oh[:, col0:col0 + 128], start=(j == 0), stop=False)
                        nc.tensor.matmul(
                            aggt[:, 128 * m:128 * (m + 1)],
                            prm["negrow"][:, 128 * m:128 * (m + 1)],
                            deg_sb[:, lo:lo + 128], start=(kf == 0), stop=True)
                        nc.vector.tensor_tensor(z_pre[m][:, lo:lo + cw],
                                                aggt[:, 128 * m:128 * m + cw],
                                                u[m][:, lo:lo + cw], op=ALU.add)
                    sp += kf
                # pass 1: chunks accumulate on top
                sp = 0
                for w in range(W):
                    kf = int(k_fix[1, w])
                    if kf == 0:
                        continue
                    lo = w * 128
                    cw = min(128, NC - lo)
                    aggt = ps_agg.tile([128, 256], FP32, tag="agg", name="aggp1")
                    for m in range(2):
                        for j in range(kf):
                            ci = sp + j
                            gsl = getg[1](ci)
                            oh, col0 = oh_maps[1](ci)
                            nc.tensor.matmul(
                                aggt[:, 128 * m:128 * (m + 1)],
                                gsl[:, 128 * m:128 * (m + 1)],
                                oh[:, col0:col0 + 128],
                                start=(j == 0), stop=(j == kf - 1))
                        nc.vector.tensor_tensor(z_pre[m][:, lo:lo + cw],
                                                aggt[:, 128 * m:128 * m + cw],
                                                z_pre[m][:, lo:lo + cw], op=ALU.add)
                    sp += kf

                dump(f"d_zpre{l}", z_pre)
                z2 = [zpool.tile([128, NC], BF16, tag=f"z2_{m}",
                                 name=f"z2_{m}_{l}") for m in range(2)]
                mlp(l, z_pre, z2, prm["w1s"], lambda k, m: (2 * k + m) * 128)
                if l == 1:
                    dump("d_z21", z2)
                stats_ar(z2)
                if l < L - 1:
                    slice_writes(z2)
                    if not skip_coll:
                        nc.gpsimd.collective_compute(
                            "AllGather", ALU.bypass, replica_groups=RG,
                            ins=[sliceT[0].opt()], outs=[tableL.opt()])
                        nc.gpsimd.collective_compute(
                            "AllGather", ALU.bypass, replica_groups=RG,
                            ins=[sliceT[1].opt()], outs=[tableH.opt()])

            # ================= final layer stats + pool + head =============
            prm = stats_params(L - 1, want_agg=False)
            u = compute_u(z2, prm["nshift_fm"])
            pooling(L - 1, u, prm["scl"])

            if not skip_coll:
                nc.gpsimd.collective_compute(
                    "AllReduce", ALU.add, replica_groups=RG,
                    ins=[arp_in.opt()], outs=[arp_out.opt()])
            y1ps = ps_mlp.tile([128, 512], FP32, tag="mlp")
            gtiles = []
            for k in range(DL // 128):
                gk = tpool.tile([128, 512], FP32, tag="gark", bufs=1)
                eng = nc.sync if k % 2 == 0 else nc.scalar
                eng.dma_start(gk[:], arp_out[128 * k:128 * (k + 1), :])
                gtiles.append(gk)
            for k in range(DL // 128):
                nc.tensor.matmul(y1ps[:], wfc1_sb[:, 128 * k:128 * (k + 1)],
                                 gtiles[k][:], start=(k == 0), stop=(k == DL // 128 - 1))
            y1 = tpool.tile([128, 512], FP32, tag="y1", bufs=1)
            nc.scalar.activation(y1[:], y1ps[:], AF.Relu, bias=bfc1_sb[:])
            y2ps = ps_msc.tile([1, 512], FP32, tag="msc")
            nc.tensor.matmul(y2ps[:], wfc2_sb[:], y1[:], start=True, stop=True)
            osb = tpool.tile([1, 512], FP32, tag="osb")
            nc.scalar.activation(osb[:], y2ps[:], AF.Identity, bias=bfc2_sb[:])
            nc.sync.dma_start(out_d[:], osb[:])

        if reps:
            with tc.For_i(0, reps, 1):
                emit_body()
        else:
            emit_body()

    nc.compile()
    return nc


# ==================================================================== run
_CACHE = {}


def _get_runner(p):
    import jax
    from jax.sharding import Mesh, PartitionSpec
    from jax.experimental.shard_map import shard_map
    from concourse.bass2jax import _bass_exec_p, install_neuronx_cc_hook

    nc = build_program(p)
    install_neuronx_cc_hook()
    part_name = nc.partition_id_tensor.name if nc.partition_id_tensor else None
    in_names, out_names, out_avals, zero_outs = [], [], [], []
    for alloc in nc.m.functions[0].allocations:
        if not isinstance(alloc, mybir.MemoryLocationSet):
            continue
        name = alloc.memorylocations[0].name
        if alloc.kind == "ExternalInput":
            if name != part_name:
                in_names.append(name)
        elif alloc.kind == "ExternalOutput":
            out_names.append(name)
            shape = tuple(alloc.tensor_shape)
            dtype = mybir.dt.np(alloc.dtype)
            out_avals.append(jax.core.ShapedArray(shape, dtype))
            zero_outs.append(np.zeros(shape, dtype))
    n_params = len(in_names)
    all_in_names = list(in_names) + list(out_names)
    if part_name is not None:
        all_in_names.append(part_name)

    def _body(*args):
        from concourse.bass2jax import partition_id_tensor
        operands = list(args)
        if part_name is not None:
            operands.append(partition_id_tensor())
        outs = _bass_exec_p.bind(
            *operands, out_avals=tuple(out_avals), in_names=tuple(all_in_names),
            out_names=tuple(out_names), lowering_input_output_aliases=(),
            sim_require_finite=False, sim_require_nnan=False, nc=nc)
        return tuple(outs)

    devices = jax.devices()[:N_CORES]
    mesh = Mesh(np.asarray(devices), ("core",))
    specs = (PartitionSpec("core"),) * (n_params + len(out_names))
    fn = jax.jit(shard_map(_body, mesh=mesh, in_specs=specs,
                           out_specs=(PartitionSpec("core"),) * len(out_names),
                           check_rep=False), keep_unused=True)
    return nc, fn, in_names, out_names, out_avals, zero_outs, mesh


def _device_args(p):
    import jax
    from jax.sharding import NamedSharding, PartitionSpec
    nc, fn, in_names, out_names, out_avals, zero_outs, mesh = _CACHE["runner"]
    per_core_maps = []
    for c in range(N_CORES):
        m = dict(p.shared)
        m.update(p.per_core[c])
        per_core_maps.append(m)
    concat_in = [np.concatenate([np.asarray(per_core_maps[c][nm])[None]
                                 for c in range(N_CORES)], axis=0)
                 .reshape(-1, *np.asarray(per_core_maps[0][nm]).shape[1:])
                 for nm in in_names]
    concat_zero = [np.zeros((N_CORES * z.shape[0], *z.shape[1:]), z.dtype)
                   for z in zero_outs]
    sh = NamedSharding(mesh, PartitionSpec("core"))
    args = [jax.device_put(a, sh) for a in concat_in + concat_zero]
    for a in args:
        a.block_until_ready()
    return args


def run_on_device(p):
    import jax
    sig = (p.N, p.E, p.G, p.Ktot, tuple(map(int, p.K_pass)),
           tuple(map(int, p.k_fix.ravel())))
    if _CACHE.get("sig") != sig:
        _CACHE.clear()
        _CACHE["sig"] = sig
    if "runner" not in _CACHE:
        _CACHE["runner"] = _get_runner(p)
    if "args" not in _CACHE:
        _CACHE["args"] = _device_args(p)
    nc, fn, in_names, out_names, out_avals, zero_outs, mesh = _CACHE["runner"]
    outs = fn(*_CACHE["args"])
    for o in outs:
        o.block_until_ready()
    res = np.asarray(outs[out_names.index("out")])
    res = res.reshape(N_CORES, 1, 512)[0, 0]     # core 0
    return res


def kernel(**inputs):
    p = preprocess(**inputs)
    _CACHE.pop("args", None)       # force fresh input upload for new data
    out = run_on_device(p)
    return out[:p.G].astype(np.float32).reshape(p.G, 1)



# revision 37
# speedup vs baseline: 2.3392x; 1.0430x over previous
"""Trainium2 Bass kernel for a 3-layer GIN-style GNN (nn_BaseGNN).

Sharding: data-parallel over nodes/edges by dst-owner across 8 NeuronCores.
Aggregation = one-hot matmuls over 128-edge chunks (PSUM accumulate), edge
source rows fetched from a replicated table in DRAM via gpsimd dma_gather.

Key structure (v2):
 - Layer 0 aggregates raw x (128-dim) and applies the encoder afterwards by
   linearity: z0 = (x_own + sum_j x_j) @ W_enc + (1+deg) b_enc.  The x table
   is a static replicated input; no initial table build or AllGather.
 - BatchNorm is algebraically folded: with s = gamma/sigma > 0,
   h = relu(s*(z - shift)) = s * relu(z - shift), so the inter-layer tables
   store RAW z2.  relu(z - shift) = max(z, shift) - shift; the max is applied
   per gathered tile, and the -shift*deg rank-1 term is injected into the
   aggregation PSUM via a 1-partition matmul with the in-degree vector.
   s is folded into the next layer's W1 (per-partition scale) and into the
   pooling output.  Hence slice writes (raw z2 transposes) do not wait for
   the BN stats AllReduce, which overlaps the table AllGathers.
 - Pooling of layer l runs inside layer l+1 (after global stats arrive),
   off the critical path.  The last layer keeps stats AllReduce + pool.

All instruction streams are identical across cores (SPMD); per-core
variation lives exclusively in input data.
"""

import numpy as np
import ml_dtypes

import concourse.bass as bass
import concourse.bacc as bacc
import concourse.mybir as mybir
import concourse.tile as tile
from concourse.masks import make_identity

BF16 = mybir.dt.bfloat16
FP32 = mybir.dt.float32
I16 = mybir.dt.int16
FP32R = mybir.dt.float32r
AF = mybir.ActivationFunctionType
ALU = mybir.AluOpType

N_CORES = 8
GI_CHUNKS = 6           # 128-edge chunks per dma_gather instruction
                        # (768 idxs = 768 descs leaves slack in the 1024-desc
                        # SWDGE queue ring; bigger gathers block on their drain)
CB = 8                 # one-hot chunks per batched is_equal
N_SWDGE_QUEUES = 4
BN_EPS = 1e-5


def _cdiv(a, b):
    return (a + b - 1) // b


class Plan:
    pass


# ==================================================================== host
def preprocess(x, edge_index, batch, num_graphs, W_enc, b_enc, W1, b1, W2, b2,
               gamma, beta, W_fc1, b_fc1, W_fc2, b_fc2):
    p = Plan()
    N, F_IN = x.shape
    D = W_enc.shape[1]
    L = W1.shape[0]
    G = int(num_graphs)
    E = edge_index.shape[1]
    C = N_CORES
    assert N % C == 0
    NC = N // C
    assert NC % 2 == 0
    HALF = NC // 2
    W = _cdiv(NC, 128)
    p.N, p.F_IN, p.D, p.L, p.G, p.E = N, F_IN, D, L, G, E
    p.NC, p.HALF, p.W = NC, HALF, W
    assert D == 256 and F_IN == 128, "layout hardcodes D=256, F_IN=128"
    assert HALF < 32768, "int16 gather index range"
    assert np.all(np.asarray(gamma) > 0), "BN fold requires gamma > 0"

    src = np.asarray(edge_index[0], np.int64)
    dst = np.asarray(edge_index[1], np.int64)
    batch = np.asarray(batch, np.int64)

    owner = dst // NC
    src_owner = src // NC
    src_local = src % NC
    src_half = (src_local >= HALF).astype(np.int64)
    table_row = HALF * src_owner + (src_local % HALF)
    dst_local = dst - owner * NC
    win = dst_local // 128

    counts = np.zeros((C, 2, W), np.int64)
    np.add.at(counts, (owner, src_half, win), 1)
    k_fix = _cdiv(counts, 128).max(axis=0)          # [2, W]
    p.k_fix = k_fix
    K_pass = k_fix.sum(axis=1).astype(np.int64)
    p.K_pass = K_pass
    p.Ktot = int(K_pass.sum())

    order = np.lexsort((dst_local, win, src_half, owner))
    so_owner = owner[order]
    so_half = src_half[order]
    so_win = win[order]
    so_row = table_row[order]
    so_dstloc = (dst_local - win * 128)[order]

    # chunk slot base per (p, w) in each pass stream
    slot_base = np.zeros((2, W), np.int64)
    for ph in range(2):
        b = 0
        for w in range(W):
            slot_base[ph, w] = b
            b += int(k_fix[ph, w]) * 128

    # per-(c,p,w) edge segment boundaries in the sorted arrays
    seg = np.zeros((C, 2, W, 2), np.int64)
    keys = ((so_owner * 2 + so_half) * W + so_win)
    bounds = np.searchsorted(keys, np.arange(C * 2 * W + 1))
    for c in range(C):
        for ph in range(2):
            for w in range(W):
                kk = (c * 2 + ph) * W + w
                seg[c, ph, w] = bounds[kk], bounds[kk + 1]

    def wrap16(lin):
        S = lin.shape[0] // 16
        t = lin.reshape(S, 16).T
        return np.ascontiguousarray(np.tile(t, (8, 1)).astype(np.int16))

    cnt = np.bincount(batch, minlength=G).astype(np.float64)
    cnt_inv = (1.0 / np.maximum(cnt, 1.0)).astype(np.float32)

    WPAD = W * 128
    xf = np.asarray(x, np.float32)
    xb_full = xf.astype(ml_dtypes.bfloat16).reshape(C, 2, HALF, F_IN)
    xtabs_bf = [np.ascontiguousarray(xb_full[:, 0].reshape(C * HALF, F_IN)),
                np.ascontiguousarray(xb_full[:, 1].reshape(C * HALF, F_IN))]
    dst_iota = np.arange(128, dtype=np.float32)
    p.per_core = []
    for c in range(C):
        d = {}
        for ph in range(2):
            K = int(K_pass[ph])
            idx = np.zeros((K * 128,), np.int16)
            dl = np.full((K * 128,), -1.0, np.float32)
            for w in range(W):
                a, b = seg[c, ph, w]
                n = b - a
                sb = int(slot_base[ph, w])
                idx[sb:sb + n] = so_row[a:b].astype(np.int16)
                dl[sb:sb + n] = so_dstloc[a:b].astype(np.float32)
            d[f"idx{ph}"] = wrap16(idx)
            # host-pregathered x rows for layer 0: [128 slot, K, 128 feat]
            xg = np.take(xtabs_bf[ph], idx.reshape(K, 128).astype(np.int64),
                         axis=0)                      # [K, 128, F_IN]
            d[f"xg{ph}"] = np.ascontiguousarray(
                xg.transpose(1, 0, 2).reshape(128, K * F_IN))
            if ph == 0:
                dl0 = dl
            else:
                dl = np.concatenate([dl0, dl])
        d["dstloc"] = np.ascontiguousarray(
            dl.reshape(p.Ktot, 128).T).astype(ml_dtypes.bfloat16)
        nb = batch[c * NC:(c + 1) * NC]
        g_lo = int(nb[0])
        span = int(nb[-1]) - g_lo + 1
        assert span <= 128, f"core {c} spans {span} graphs"
        bl = np.full((W * 128,), -1.0, np.float32)
        bl[:NC] = (nb - g_lo).astype(np.float32)
        blm = bl.reshape(W, 128).T                    # [128 node, W]
        # pooling one-hot const: pooh[p, j, d] = (batchloc[p,j] == d)
        d["pooh"] = np.ascontiguousarray(
            (blm[:, :, None] == dst_iota[None, None, :])
            .astype(ml_dtypes.bfloat16).reshape(128, W * 128))
        pl = np.zeros((128, 512), np.float32)
        hi = min(128, G - g_lo)
        pl[np.arange(hi), g_lo + np.arange(hi)] = cnt_inv[g_lo:g_lo + hi]
        d["placem"] = pl.astype(ml_dtypes.bfloat16)
        d["xTown"] = np.ascontiguousarray(
            xf[c * NC:(c + 1) * NC].T).astype(ml_dtypes.bfloat16)
        degc = np.bincount(dst_local[owner == c], minlength=NC).astype(np.float32)
        dv = np.zeros((1, WPAD), np.float32)
        dv[0, :NC] = degc
        d["deg"] = dv.astype(ml_dtypes.bfloat16)
        p.per_core.append(d)

    sh = {}
    sh["iota"] = np.tile(np.arange(128, dtype=np.float32),
                         (128, CB)).astype(ml_dtypes.bfloat16)
    sh["wenc"] = np.asarray(W_enc, np.float32).astype(ml_dtypes.bfloat16)
    w1c = np.zeros((128, L * 4 * 128), np.float32)
    w2c = np.zeros((128, L * 4 * 128), np.float32)
    for l in range(L):
        for k in range(2):
            for m in range(2):
                col = ((l * 2 + k) * 2 + m) * 128
                w1c[:, col:col + 128] = W1[l, 128 * k:128 * (k + 1), 128 * m:128 * (m + 1)]
                w2c[:, col:col + 128] = W2[l, 128 * k:128 * (k + 1), 128 * m:128 * (m + 1)]
    sh["w1"] = w1c
    sh["w2"] = w2c

    def col2(v):
        return np.ascontiguousarray(np.asarray(v, np.float32).reshape(2, 128).T)

    sh["bencc"] = col2(b_enc)
    sh["bencrow"] = np.asarray(b_enc, np.float32).reshape(1, 256).astype(
        ml_dtypes.bfloat16)
    p.benc_nonzero = bool(np.any(np.asarray(b_enc) != 0))
    sh["b1"] = np.concatenate([col2(b1[l]) for l in range(L)], axis=1)
    sh["b2"] = np.concatenate([col2(b2[l]) for l in range(L)], axis=1)
    sh["gaminv"] = np.concatenate(
        [col2(1.0 / np.asarray(gamma[l], np.float64)) for l in range(L)], axis=1)
    sh["bet"] = np.concatenate([col2(beta[l]) for l in range(L)], axis=1)
    DL = D * L
    sh["wfc1"] = np.ascontiguousarray(
        np.asarray(W_fc1, np.float32).reshape(DL // 128, 128, 128)
        .transpose(1, 0, 2).reshape(128, DL))
    sh["wfc2"] = np.asarray(W_fc2, np.float32).reshape(128, 1)
    sh["bfc1"] = np.asarray(b_fc1, np.float32).reshape(128, 1)
    sh["bfc2"] = np.asarray(b_fc2, np.float32).reshape(1, 1)
    p.shared = sh
    return p


# ================================================================= program
def build_program(p, reps=0, skip_coll=False, dbg=False):
    C = N_CORES
    N, D, L, W, NC, HALF = p.N, p.D, p.L, p.W, p.NC, p.HALF
    NROW = C * HALF
    K_pass, k_fix, Ktot = p.K_pass, p.k_fix, p.Ktot
    NCH = _cdiv(NC, 128)
    MJ = _cdiv(NC, 512)
    DL = D * L
    WPAD = W * 128

    nc = bacc.Bacc("TRN2", target_bir_lowering=False, debug=False,
                   enable_asserts=False, num_devices=C,
                   num_swdge_queues=N_SWDGE_QUEUES)

    ein = {}

    def EIN(name, shape, dt):
        ein[name] = nc.dram_tensor(name, list(shape), dt, kind="ExternalInput").ap()
        return ein[name]

    xTown = EIN("xTown", [128, NC], BF16)
    idx_in = [EIN("idx0", [128, int(K_pass[0]) * 8], I16),
              EIN("idx1", [128, int(K_pass[1]) * 8], I16)]
    xg_in = [EIN("xg0", [128, int(K_pass[0]) * 128], BF16),
             EIN("xg1", [128, int(K_pass[1]) * 128], BF16)]
    dstloc_in = EIN("dstloc", [128, Ktot], BF16)
    iota_in = EIN("iota", [128, CB * 128], BF16)
    pooh_in = EIN("pooh", [128, W * 128], BF16)
    placem_in = EIN("placem", [128, 512], BF16)
    deg_in = EIN("deg", [1, WPAD], BF16)
    wenc_in = EIN("wenc", [128, 256], BF16)
    w1_in = EIN("w1", [128, L * 4 * 128], FP32R)
    w2_in = EIN("w2", [128, L * 4 * 128], FP32R)
    bencc_in = EIN("bencc", [128, 2], FP32)
    bencrow_in = EIN("bencrow", [1, 256], BF16)
    b1_in = EIN("b1", [128, L * 2], FP32)
    b2_in = EIN("b2", [128, L * 2], FP32)
    gaminv_in = EIN("gaminv", [128, L * 2], FP32)
    bet_in = EIN("bet", [128, L * 2], FP32)
    wfc1_in = EIN("wfc1", [128, DL], FP32)
    wfc2_in = EIN("wfc2", [128, 1], FP32)
    bfc1_in = EIN("bfc1", [128, 1], FP32)
    bfc2_in = EIN("bfc2", [1, 1], FP32)

    out_d = nc.dram_tensor("out", [1, 512], FP32, kind="ExternalOutput").ap()
    dbg_t = {}
    if dbg:
        for nm, shape, dt_ in [
                ("d_xsum", [128, NC], BF16), ("d_zpre0", [128, 2 * NC], FP32),
                ("d_z20", [128, 2 * NC], BF16), ("d_u0", [128, 2 * NC], BF16),
                ("d_mrep", [128, 256], BF16), ("d_zpre1", [128, 2 * NC], FP32),
                ("d_z21", [128, 2 * NC], BF16), ("d_zpre2", [128, 2 * NC], FP32)]:
            dbg_t[nm] = nc.dram_tensor(nm, shape, dt_,
                                       kind="ExternalOutput").ap()

    tableL = nc.dram_tensor("tableL", [NROW, 256], BF16, kind="Internal").ap()
    tableH = nc.dram_tensor("tableH", [NROW, 256], BF16, kind="Internal").ap()
    sliceT = [nc.dram_tensor(f"slice{h}", [HALF, 256], BF16, kind="Internal").ap()
              for h in range(2)]
    arbn_in = nc.dram_tensor("arbn_in", [128, 4], FP32, kind="Internal").ap()
    arbn_out = nc.dram_tensor("arbn_out", [128, 4], FP32, kind="Internal",
                              addr_space="Shared").ap()
    arp_in = nc.dram_tensor("arp_in", [DL, 512], FP32, kind="Internal").ap()
    arp_out = nc.dram_tensor("arp_out", [DL, 512], FP32, kind="Internal",
                             addr_space="Shared").ap()

    RG = [list(range(C))]
    half_t = [tableL, tableH]

    import contextlib
    with tile.TileContext(nc) as tc, contextlib.ExitStack() as ctx:
        consts = ctx.enter_context(tc.tile_pool(name="consts", bufs=1))
        gpool_s = ctx.enter_context(tc.tile_pool(name="gP", bufs=9))
        gpool = [gpool_s, gpool_s]
        ohpool = ctx.enter_context(tc.tile_pool(name="oh", bufs=4))
        zpool = ctx.enter_context(tc.tile_pool(name="z", bufs=1))
        spool = ctx.enter_context(tc.tile_pool(name="s", bufs=2))
        tpool = ctx.enter_context(tc.tile_pool(name="t", bufs=2))
        ppool = ctx.enter_context(tc.tile_pool(name="prm", bufs=1))
        ps_agg = ctx.enter_context(tc.tile_pool(name="ps_agg", bufs=3, space="PSUM"))
        ps_mlp = ctx.enter_context(tc.tile_pool(name="ps_mlp", bufs=2, space="PSUM"))
        ps_msc = ctx.enter_context(tc.tile_pool(name="ps_msc", bufs=2, space="PSUM"))

        def load_const(apin, shape, dt):
            t = consts.tile(shape, dt, name="c_" + apin.tensor.name)
            nc.sync.dma_start(t[:], apin[:])
            return t

        pooh_sb = load_const(pooh_in, [128, W * 128], BF16)
        iota_sb = load_const(iota_in, [128, CB * 128], BF16)
        wenc_sb = load_const(wenc_in, [128, 256], BF16)
        w1_sb = load_const(w1_in, [128, L * 4 * 128], FP32R)
        w2_sb = load_const(w2_in, [128, L * 4 * 128], FP32R)
        bencc_sb = load_const(bencc_in, [128, 2], FP32)
        bencrow_sb = load_const(bencrow_in, [1, 256], BF16)
        b1_sb = load_const(b1_in, [128, L * 2], FP32)
        b2_sb = load_const(b2_in, [128, L * 2], FP32)
        gaminv_sb = load_const(gaminv_in, [128, L * 2], FP32)
        bet_sb = load_const(bet_in, [128, L * 2], FP32)
        wfc1_sb = load_const(wfc1_in, [128, DL], FP32)
        wfc2_sb = load_const(wfc2_in, [128, 1], FP32)
        bfc1_sb = load_const(bfc1_in, [128, 1], FP32)
        bfc2_sb = load_const(bfc2_in, [1, 1], FP32)
        dstloc_sb = load_const(dstloc_in, [128, Ktot], BF16)
        placem_sb = load_const(placem_in, [128, 512], BF16)
        deg_sb = load_const(deg_in, [1, WPAD], BF16)
        idx_sb = [load_const(idx_in[0], [128, int(K_pass[0]) * 8], I16),
                  load_const(idx_in[1], [128, int(K_pass[1]) * 8], I16)]
        ident = consts.tile([128, 128], BF16)
        make_identity(nc, ident[:])
        ones_col = consts.tile([1, 128], BF16)
        nc.vector.memset(ones_col[:], 1.0)

        n_ginst = [int(_cdiv(int(K_pass[ph]), GI_CHUNKS)) for ph in range(2)]

        def emit_body():
            # ---------- gathers helper ------------------------------------
            def emit_gathers(ph, table_ap, es):
                tiles = []
                K = int(K_pass[ph])
                for i in range(n_ginst[ph]):
                    c0 = i * GI_CHUNKS
                    nch = min(GI_CHUNKS, K - c0)
                    g = gpool[ph].tile([128, GI_CHUNKS, es], BF16, tag="g",
                                       name=f"g{ph}t")
                    nc.gpsimd.dma_gather(
                        g[:, :nch, :], table_ap[:], idx_sb[ph][:, c0 * 8:(c0 + nch) * 8],
                        num_idxs=nch * 128, num_idxs_reg=nch * 128, elem_size=es,
                        single_packet=False, queue_num=(i % N_SWDGE_QUEUES),
                    )
                    tiles.append(g)
                return tiles

            class GetG:
                """Lazily applies max(g, shift) (u-space relu prep) the first
                time a gather tile is touched, in consumption order, so the
                DVE stream order matches the window loops (no queue cycle)."""
                def __init__(self, ph, tiles, mrep):
                    self.ph, self.tiles, self.mrep = ph, tiles, mrep
                    self.maxed = set()

                def __call__(self, ci):
                    gi = ci // GI_CHUNKS
                    g = self.tiles[gi]
                    if self.mrep is not None and gi not in self.maxed:
                        nch = min(GI_CHUNKS, int(K_pass[self.ph]) - gi * GI_CHUNKS)
                        nc.vector.tensor_tensor(
                            g[:, :nch, :], g[:, :nch, :],
                            self.mrep[:].unsqueeze(1).to_broadcast([128, nch, 256]),
                            op=ALU.max)
                        self.maxed.add(gi)
                    return g[:, ci % GI_CHUNKS, :]

            class OhLazy:
                """Loads precomputed one-hot groups on first touch, in
                consumption order.  base = ohs column of this pass's chunk 0."""
                def __init__(self, base, K):
                    self.base, self.K = base, K
                    self.map = {}

                def __call__(self, ci):
                    if ci not in self.map:
                        g0 = ci - ci % CB
                        cn = min(CB, self.K - g0)
                        oh = ohpool.tile([128, CB * 128], BF16, tag="oh",
                                         name="oht")
                        c0 = self.base + g0
                        nc.vector.tensor_tensor(
                            oh[:, :cn * 128], iota_sb[:, :cn * 128],
                            dstloc_sb[:, c0:c0 + cn].to_broadcast([128, cn, 128]),
                            op=ALU.is_equal)
                        for k in range(cn):
                            self.map[g0 + k] = (oh, k * 128)
                    return self.map[ci]

            # ---------- shared MLP (z_pre -> z2), feature-major ------------
            def mlp_jj(l, jj, z_pre, z2, w1src, w1cols):
                lo = jj * 512
                nw = min(512, NC - lo)
                z1t = [tpool.tile([128, 512], FP32R, tag=f"z1_{m}", bufs=1,
                                  name=f"z1t{m}") for m in range(2)]
                for m in range(2):
                    ps = ps_mlp.tile([128, 512], FP32, tag="mlp")
                    for k in range(2):
                        col = w1cols(k, m)
                        nc.tensor.matmul(ps[:, :nw],
                                         w1src[:, col:col + 128],
                                         z_pre[k][:, lo:lo + nw],
                                         start=(k == 0), stop=(k == 1))
                    nc.scalar.activation(z1t[m][:, :nw], ps[:, :nw], AF.Relu,
                                         bias=b1_sb[:, 2 * l + m:2 * l + m + 1])
                for m in range(2):
                    ps = ps_mlp.tile([128, 512], FP32, tag="mlp")
                    for k in range(2):
                        col = ((l * 2 + k) * 2 + m) * 128
                        nc.tensor.matmul(ps[:, :nw],
                                         w2_sb[:, col:col + 128],
                                         z1t[k][:, :nw],
                                         start=(k == 0), stop=(k == 1))
                    nc.scalar.activation(z2[m][:, lo:lo + nw], ps[:, :nw],
                                         AF.Identity,
                                         bias=b2_sb[:, 2 * l + m:2 * l + m + 1])

            # ---------- BN stats of z2 -> small AllReduce (split) ----------
            def stats_start():
                return [tpool.tile([128, MJ, 6], FP32, tag=f"bnacc{m}", bufs=1,
                                   name=f"bnacc{m}") for m in range(2)]

            def stats_accum(bnacc, z2, jj):
                lo = jj * 512
                nw = min(512, NC - lo)
                for m in range(2):
                    nc.vector.bn_stats(bnacc[m][:, jj, :], z2[m][:, lo:lo + nw])

            def stats_finish(bnacc):
                stt = spool.tile([128, 4], FP32, tag="stt")
                for m in range(2):
                    ag = spool.tile([128, 2], FP32, tag="bnag")
                    nc.vector.bn_aggr(ag[:], bnacc[m][:])
                    sq = spool.tile([128, 1], FP32, tag="bnsq")
                    nc.vector.tensor_tensor(sq[:], ag[:, 0:1], ag[:, 0:1],
                                            op=ALU.mult)
                    nc.vector.tensor_copy(stt[:, 2 * m:2 * m + 1], ag[:, 0:1])
                    nc.vector.tensor_tensor(stt[:, 2 * m + 1:2 * m + 2], ag[:, 1:2],
                                            sq[:], op=ALU.add)
                nc.sync.dma_start(arbn_in[:], stt[:])
                if not skip_coll:
                    nc.gpsimd.collective_compute(
                        "AllReduce", ALU.add, replica_groups=RG,
                        ins=[arbn_in.opt()], outs=[arbn_out.opt()])

            # ---------- stats -> scl / shift / rows / m_rep / w1s ----------
            def stats_params(l, want_agg):
                """Consume arbn_out holding layer-l stats.  Returns dict with
                scl [128,2] f32, nshift_fm [128,2] f32 (= -shift), and if
                want_agg: m_rep [128,256] bf16, negrow [1,256] bf16, w1s."""
                stg = spool.tile([128, 4], FP32, tag="stg")
                nc.sync.dma_start(stg[:], arbn_out[:])
                scl = ppool.tile([128, 2], FP32, tag="scl", name="scl")
                shift = ppool.tile([128, 2], FP32, tag="shift", name="shift")
                for m in range(2):
                    mean = spool.tile([128, 1], FP32, tag="bmean")
                    e2 = spool.tile([128, 1], FP32, tag="be2")
                    nc.vector.tensor_scalar(mean[:], stg[:, 2 * m:2 * m + 1],
                                            1.0 / C, None, op0=ALU.mult)
                    nc.vector.tensor_scalar(e2[:], stg[:, 2 * m + 1:2 * m + 2],
                                            1.0 / C, None, op0=ALU.mult)
                    var = spool.tile([128, 1], FP32, tag="bvar")
                    nc.vector.tensor_tensor(var[:], mean[:], mean[:], op=ALU.mult)
                    nc.vector.tensor_tensor(var[:], e2[:], var[:], op=ALU.subtract)
                    nc.vector.tensor_scalar(var[:], var[:], float(BN_EPS), None,
                                            op0=ALU.add)
                    sd = spool.tile([128, 1], FP32, tag="bsd")
                    nc.scalar.activation(sd[:], var[:], AF.Sqrt)
                    inv_s = spool.tile([128, 1], FP32, tag="binv")
                    nc.vector.tensor_tensor(inv_s[:], sd[:],
                                            gaminv_sb[:, 2 * l + m:2 * l + m + 1],
                                            op=ALU.mult)
                    nc.vector.reciprocal(scl[:, m:m + 1], inv_s[:])
                    tmp = spool.tile([128, 1], FP32, tag="btmp")
                    nc.vector.tensor_tensor(tmp[:], inv_s[:],
                                            bet_sb[:, 2 * l + m:2 * l + m + 1],
                                            op=ALU.mult)
                    nc.vector.tensor_tensor(shift[:, m:m + 1],
                                            mean[:], tmp[:], op=ALU.subtract)
                nshift_fm = ppool.tile([128, 2], FP32, tag="nshf", name="nshf")
                nc.vector.tensor_scalar(nshift_fm[:], shift[:], -1.0, None,
                                        op0=ALU.mult)
                prm = {"scl": scl, "nshift_fm": nshift_fm}
                if want_agg:
                    shift_bf = spool.tile([128, 2], BF16, tag="shbf")
                    nc.vector.tensor_copy(shift_bf[:], shift[:])
                    shrow = ppool.tile([1, 256], BF16, tag="shrow", name="shrow")
                    negrow = ppool.tile([1, 256], BF16, tag="negrow", name="negrow")
                    for m in range(2):
                        rps = ps_msc.tile([1, 128], FP32, tag="msc")
                        nc.tensor.matmul(rps[:], shift_bf[:, m:m + 1], ident[:],
                                         start=True, stop=True)
                        nc.vector.tensor_copy(shrow[:, 128 * m:128 * (m + 1)], rps[:])
                        nc.vector.tensor_scalar(negrow[:, 128 * m:128 * (m + 1)],
                                                rps[:], -1.0, None, op0=ALU.mult)
                    mps = ps_msc.tile([128, 256], FP32, tag="msc")
                    nc.tensor.matmul(mps[:], ones_col[:], shrow[:],
                                     start=True, stop=True)
                    m_rep = ppool.tile([128, 256], BF16, tag="mrep", name="mrep")
                    nc.vector.tensor_copy(m_rep[:], mps[:])
                    w1s = ppool.tile([128, 512], FP32R, tag="w1s", name="w1s")
                    lw = l + 1
                    for k in range(2):
                        for m in range(2):
                            col = ((lw * 2 + k) * 2 + m) * 128
                            nc.scalar.activation(
                                w1s[:, (2 * k + m) * 128:(2 * k + m + 1) * 128],
                                w1_sb[:, col:col + 128], AF.Identity,
                                scale=scl[:, k:k + 1])
                    prm.update(m_rep=m_rep, negrow=negrow, w1s=w1s)
                return prm

            # ---------- u_own = relu(z2 - shift), in place on z2 -----------
            # (z2's other consumers - stats, slice transposes - precede this)
            def compute_u(z2, nshift_fm):
                for m in range(2):
                    for jj in range(MJ):
                        lo = jj * 512
                        nw = min(512, NC - lo)
                        nc.scalar.activation(z2[m][:, lo:lo + nw],
                                             z2[m][:, lo:lo + nw], AF.Relu,
                                             bias=nshift_fm[:, m:m + 1])
                return z2

            # ---------- pooling of layer l (h = scl * u) -------------------
            def pooling(l, u, scl):
                pooled_ps = ps_msc.tile([128, 256], FP32, tag="pool", bufs=1)
                for j in range(NCH):
                    lo = j * 128
                    cw = min(128, NC - lo)
                    unm = tpool.tile([128, 256], BF16, tag="unm", bufs=1)
                    for m in range(2):
                        tp = ps_msc.tile([128, 128], BF16, tag="msc")
                        nc.tensor.transpose(tp[:cw, :], u[m][:, lo:lo + cw],
                                            ident[:])
                        nc.vector.tensor_copy(unm[:cw, 128 * m:128 * (m + 1)],
                                              tp[:cw, :])
                    nc.tensor.matmul(pooled_ps[:],
                                     pooh_sb[:cw, j * 128:(j + 1) * 128],
                                     unm[:cw, :],
                                     start=(j == 0), stop=(j == NCH - 1))
                pooled_sb = tpool.tile([128, 256], BF16, tag="pooled", bufs=1)
                nc.vector.tensor_copy(pooled_sb[:], pooled_ps[:])
                for m in range(2):
                    pl_ps = ps_msc.tile([128, 512], FP32, tag="msc")
                    nc.tensor.matmul(pl_ps[:], pooled_sb[:, 128 * m:128 * (m + 1)],
                                     placem_sb[:], start=True, stop=True)
                    gp = tpool.tile([128, 512], FP32, tag="gp", bufs=1)
                    nc.scalar.activation(gp[:], pl_ps[:], AF.Identity,
                                         scale=scl[:, m:m + 1])
                    nc.scalar.dma_start(arp_in[(l * 2 + m) * 128:(l * 2 + m + 1) * 128, :],
                                        gp[:])

            # ---------- raw-z2 transposes -> slice writes ------------------
            def slice_writes_win(z2, j):
                lo = j * 128
                cw = min(128, NC - lo)
                znm = tpool.tile([128, 256], BF16, tag="znm")
                for m in range(2):
                    tp = ps_msc.tile([128, 128], BF16, tag="msc")
                    nc.tensor.transpose(tp[:cw, :], z2[m][:, lo:lo + cw],
                                        ident[:])
                    nc.vector.tensor_copy(znm[:cw, 128 * m:128 * (m + 1)],
                                          tp[:cw, :])
                for (a, b) in ((lo, min(lo + cw, HALF)), (max(lo, HALF), lo + cw)):
                    if b <= a:
                        continue
                    hh = 0 if a < HALF else 1
                    r0 = a - hh * HALF
                    nc.scalar.dma_start(sliceT[hh][r0:r0 + (b - a), :],
                                        znm[a - lo:b - lo, :])

            # ---------- per-jj tail: MLP + stats + slices, interleaved -----
            class TailFlush:
                """As pass-1 windows finalize z_pre, flush completed 512-col
                jj chunks through the MLP / BN-stats / slice-write tail so
                that tail work overlaps the remaining aggregation."""
                def __init__(self, l, z_pre, z2, w1src, w1cols, do_slices,
                             pre=None):
                    self.l, self.z_pre, self.z2 = l, z_pre, z2
                    self.w1src, self.w1cols = w1src, w1cols
                    self.do_slices, self.pre = do_slices, pre
                    self.bnacc = stats_start()
                    self.next_jj = 0

                def window_done(self, w):
                    while (self.next_jj < MJ and
                           (min((self.next_jj + 1) * 512, NC) - 1) // 128 <= w):
                        jj = self.next_jj
                        if self.pre is not None:
                            self.pre(jj)
                        mlp_jj(self.l, jj, self.z_pre, self.z2,
                               self.w1src, self.w1cols)
                        stats_accum(self.bnacc, self.z2, jj)
                        if self.do_slices:
                            for j in range(jj * 4, min(jj * 4 + 4, NCH)):
                                slice_writes_win(self.z2, j)
                        self.next_jj += 1

                def finish(self):
                    assert self.next_jj == MJ
                    stats_finish(self.bnacc)

            # ---------- L0 x stream: host-pregathered rows, plain DMA ------
            def emit_xg_loads(ph):
                tiles = []
                K = int(K_pass[ph])
                for i in range(n_ginst[ph]):
                    c0 = i * GI_CHUNKS
                    nch = min(GI_CHUNKS, K - c0)
                    g = gpool[ph].tile([128, GI_CHUNKS, 128], BF16, tag="gx",
                                       bufs=4, name=f"gx{ph}t")
                    # sync queue ONLY: putting these on scalar would deadlock
                    # (loads would sit ahead of the flush's z1/z2 activations
                    # while waiting on PE progress that needs those acts)
                    nc.sync.dma_start(g[:, :nch, :],
                                      xg_in[ph][:, c0 * 128:(c0 + nch) * 128])
                    tiles.append(g)
                return tiles

            def dump(nm, tiles):
                if not dbg:
                    return
                if not isinstance(tiles, list):
                    tiles = [tiles]
                for m, t in enumerate(tiles):
                    tt = t[:]
                    if tt.dtype == FP32R:
                        tt = tt.bitcast(FP32)
                    nc.sync.dma_start(
                        dbg_t[nm][:, m * NC:(m + 1) * NC] if len(tiles) > 1
                        else dbg_t[nm][:, :], tt)

            # ================= LAYER 0: x-space agg + encoder ==============
            # xsum starts as x_own (loaded in place) and is accumulated into;
            # z2 gets its own tags so the flush never waits on xsum's death
            gt = [emit_xg_loads(0), emit_xg_loads(1)]
            getg = [GetG(0, gt[0], None), GetG(1, gt[1], None)]
            oh_maps = [OhLazy(0, int(K_pass[0])),
                       OhLazy(int(K_pass[0]), int(K_pass[1]))]
            xsum = zpool.tile([128, NC], BF16, tag="z2_1", name="xsum")
            nc.sync.dma_start(xsum[:], xTown[:])
            z_pre = [zpool.tile([128, NC], FP32R, tag=f"zpre{m}", name=f"zpre{m}")
                     for m in range(2)]
            z2 = [zpool.tile([128, NC], BF16, tag="z2_0", name="z2_0"),
                  zpool.tile([128, NC], BF16, tag="z2n", name="z2_1n")]

            # encoder on aggregated x: z_pre0 = xsum @ W_enc (+ bias terms)
            def enc_jj(jj):
                lo = jj * 512
                nw = min(512, NC - lo)
                for m in range(2):
                    ps = ps_mlp.tile([128, 512], FP32, tag="mlp")
                    last = not p.benc_nonzero
                    nc.tensor.matmul(ps[:, :nw],
                                     wenc_sb[:, 128 * m:128 * (m + 1)],
                                     xsum[:, lo:lo + nw], start=True, stop=last)
                    if p.benc_nonzero:
                        # + b_enc (x) deg: the deg part of (1+deg) b_enc; the
                        # +1 part comes via the activation bias below.
                        nc.tensor.matmul(ps[:, :nw],
                                         bencrow_sb[:, 128 * m:128 * (m + 1)],
                                         deg_sb[:, lo:lo + nw],
                                         start=False, stop=True)
                    nc.scalar.activation(z_pre[m][:, lo:lo + nw], ps[:, :nw],
                                         AF.Identity,
                                         bias=bencc_sb[:, m:m + 1])

            tf = TailFlush(0, z_pre, z2, w1_sb,
                           lambda k, m: ((0 * 2 + k) * 2 + m) * 128,
                           do_slices=True, pre=enc_jj)
            # pass 0: all pass-0 windows first (their gathers arrive first)
            sp = 0
            for w in range(W):
                lo = w * 128
                cw = min(128, NC - lo)
                kf = int(k_fix[0, w])
                if kf == 0:
                    continue
                aggt = ps_agg.tile([128, 256], FP32, tag="agg", name="aggx")
                agg = aggt[:, 0:128]
                for j in range(kf):
                    ci = sp + j
                    gsl = getg[0](ci)
                    oh, col0 = oh_maps[0](ci)
                    nc.tensor.matmul(agg[:, :], gsl, oh[:, col0:col0 + 128],
                                     start=(j == 0), stop=(j == kf - 1))
                nc.vector.tensor_tensor(xsum[:, lo:lo + cw], agg[:, :cw],
                                        xsum[:, lo:lo + cw], op=ALU.add)
                sp += kf
            # pass 1: finalize xsum per window, flushing the encoder/MLP tail
            sp = 0
            for w in range(W):
                lo = w * 128
                cw = min(128, NC - lo)
                kf = int(k_fix[1, w])
                if kf > 0:
                    aggt = ps_agg.tile([128, 256], FP32, tag="agg", name="aggx1")
                    agg = aggt[:, 0:128]
                    for j in range(kf):
                        ci = sp + j
                        gsl = getg[1](ci)
                        oh, col0 = oh_maps[1](ci)
                        nc.tensor.matmul(agg[:, :], gsl, oh[:, col0:col0 + 128],
                                         start=(j == 0), stop=(j == kf - 1))
                    nc.vector.tensor_tensor(xsum[:, lo:lo + cw], agg[:, :cw],
                                            xsum[:, lo:lo + cw], op=ALU.add)
                    sp += kf
                tf.window_done(w)
            tf.finish()
            dump("d_xsum", xsum)
            dump("d_zpre0", z_pre)
            dump("d_z20", z2)
            if not skip_coll:
                nc.gpsimd.collective_compute(
                    "AllGather", ALU.bypass, replica_groups=RG,
                    ins=[sliceT[0].opt()], outs=[tableL.opt()])
                nc.gpsimd.collective_compute(
                    "AllGather", ALU.bypass, replica_groups=RG,
                    ins=[sliceT[1].opt()], outs=[tableH.opt()])

            # ================= LAYERS 1..L-1 ===============================
            for l in range(1, L):
                prm = stats_params(l - 1, want_agg=True)
                u = compute_u(z2, prm["nshift_fm"])
                if l == 1:
                    dump("d_u0", u)
                    if dbg:
                        nc.sync.dma_start(dbg_t["d_mrep"][:, :],
                                          prm["m_rep"][:])
                pooling(l - 1, u, prm["scl"])

                gt = [emit_gathers(0, half_t[0], 256),
                      emit_gathers(1, half_t[1], 256)]
                getg = [GetG(0, gt[0], prm["m_rep"]), GetG(1, gt[1], prm["m_rep"])]
                oh_maps = [OhLazy(0, int(K_pass[0])),
                           OhLazy(int(K_pass[0]), int(K_pass[1]))]
                z_pre = [zpool.tile([128, NC], FP32R, tag=f"zpre{m}",
                                    name=f"zpre{m}_{l}") for m in range(2)]
                # pass 0: chunks + rank-1 (-shift x deg), then + u_own
                sp = 0
                for w in range(W):
                    lo = w * 128
                    cw = min(128, NC - lo)
                    kf = int(k_fix[0, w])
                    aggt = ps_agg.tile([128, 256], FP32, tag="agg", name="aggp0")
                    # PSUM accumulation chains must not interleave within a
                    # tile: run the m=0 chain to completion, then m=1.
                    for m in range(2):
                        for j in range(kf):
                            ci = sp + j
                            gsl = getg[0](ci)
                            oh, col0 = oh_maps[0](ci)
                            nc.tensor.matmul(
                                aggt[:, 128 * m:128 * (m + 1)],
                                gsl[:, 128 * m:128 * (m + 1)],
                                oh[:, col0:col0 + 128], start=(j == 0), stop=False)
                        nc.tensor.matmul(
                            aggt[:, 128 * m:128 * (m + 1)],
                            prm["negrow"][:, 128 * m:128 * (m + 1)],
                            deg_sb[:, lo:lo + 128], start=(kf == 0), stop=True)
                        nc.vector.tensor_tensor(z_pre[m][:, lo:lo + cw],
                                                aggt[:, 128 * m:128 * m + cw],
                                                u[m][:, lo:lo + cw], op=ALU.add)
                    sp += kf
                # pass 1: chunks accumulate on top; flush MLP/stats/slices
                # for each completed 512-col chunk so the tail overlaps agg
                z2 = [zpool.tile([128, NC], BF16, tag="z2_0",
                                 name=f"z2_0_{l}"),
                      zpool.tile([128, NC], BF16, tag="z2n",
                                 name=f"z2_1n_{l}")]
                tf = TailFlush(l, z_pre, z2, prm["w1s"],
                               lambda k, m: (2 * k + m) * 128,
                               do_slices=(l < L - 1))
                sp = 0
                for w in range(W):
                    lo = w * 128
                    cw = min(128, NC - lo)
                    kf = int(k_fix[1, w])
                    if kf > 0:
                        aggt = ps_agg.tile([128, 256], FP32, tag="agg",
                                           name="aggp1")
                        for m in range(2):
                            for j in range(kf):
                                ci = sp + j
                                gsl = getg[1](ci)
                                oh, col0 = oh_maps[1](ci)
                                nc.tensor.matmul(
                                    aggt[:, 128 * m:128 * (m + 1)],
                                    gsl[:, 128 * m:128 * (m + 1)],
                                    oh[:, col0:col0 + 128],
                                    start=(j == 0), stop=(j == kf - 1))
                            nc.vector.tensor_tensor(z_pre[m][:, lo:lo + cw],
                                                    aggt[:, 128 * m:128 * m + cw],
                                                    z_pre[m][:, lo:lo + cw],
                                                    op=ALU.add)
                        sp += kf
                    tf.window_done(w)
                tf.finish()
                dump(f"d_zpre{l}", z_pre)
                if l == 1:
                    dump("d_z21", z2)
                if l < L - 1:
                    if not skip_coll:
                        nc.gpsimd.collective_compute(
                            "AllGather", ALU.bypass, replica_groups=RG,
                            ins=[sliceT[0].opt()], outs=[tableL.opt()])
                        nc.gpsimd.collective_compute(
                            "AllGather", ALU.bypass, replica_groups=RG,
                            ins=[sliceT[1].opt()], outs=[tableH.opt()])

            # ================= final layer stats + pool + head =============
            prm = stats_params(L - 1, want_agg=False)
            u = compute_u(z2, prm["nshift_fm"])
            pooling(L - 1, u, prm["scl"])

            if not skip_coll:
                nc.gpsimd.collective_compute(
                    "AllReduce", ALU.add, replica_groups=RG,
                    ins=[arp_in.opt()], outs=[arp_out.opt()])
            y1ps = ps_mlp.tile([128, 512], FP32, tag="mlp")
            gtiles = []
            for k in range(DL // 128):
                gk = tpool.tile([128, 512], FP32, tag="gark", bufs=1)
                eng = nc.sync if k % 2 == 0 else nc.scalar
                eng.dma_start(gk[:], arp_out[128 * k:128 * (k + 1), :])
                gtiles.append(gk)
            for k in range(DL // 128):
                nc.tensor.matmul(y1ps[:], wfc1_sb[:, 128 * k:128 * (k + 1)],
                                 gtiles[k][:], start=(k == 0), stop=(k == DL // 128 - 1))
            y1 = tpool.tile([128, 512], FP32, tag="y1", bufs=1)
            nc.scalar.activation(y1[:], y1ps[:], AF.Relu, bias=bfc1_sb[:])
            y2ps = ps_msc.tile([1, 512], FP32, tag="msc")
            nc.tensor.matmul(y2ps[:], wfc2_sb[:], y1[:], start=True, stop=True)
            osb = tpool.tile([1, 512], FP32, tag="osb")
            nc.scalar.activation(osb[:], y2ps[:], AF.Identity, bias=bfc2_sb[:])
            nc.sync.dma_start(out_d[:], osb[:])

        if reps:
            with tc.For_i(0, reps, 1):
                emit_body()
        else:
            emit_body()

    nc.compile()
    return nc


# ==================================================================== run
_CACHE = {}


def _get_runner(p):
    import jax
    from jax.sharding import Mesh, PartitionSpec
    from jax.experimental.shard_map import shard_map
    from concourse.bass2jax import _bass_exec_p, install_neuronx_cc_hook

    nc = build_program(p)
    install_neuronx_cc_hook()
    part_name = nc.partition_id_tensor.name if nc.partition_id_tensor else None
    in_names, out_names, out_avals, zero_outs = [], [], [], []
    for alloc in nc.m.functions[0].allocations:
        if not isinstance(alloc, mybir.MemoryLocationSet):
            continue
        name = alloc.memorylocations[0].name
        if alloc.kind == "ExternalInput":
            if name != part_name:
                in_names.append(name)
        elif alloc.kind == "ExternalOutput":
            out_names.append(name)
            shape = tuple(alloc.tensor_shape)
            dtype = mybir.dt.np(alloc.dtype)
            out_avals.append(jax.core.ShapedArray(shape, dtype))
            zero_outs.append(np.zeros(shape, dtype))
    n_params = len(in_names)
    all_in_names = list(in_names) + list(out_names)
    if part_name is not None:
        all_in_names.append(part_name)

    def _body(*args):
        from concourse.bass2jax import partition_id_tensor
        operands = list(args)
        if part_name is not None:
            operands.append(partition_id_tensor())
        outs = _bass_exec_p.bind(
            *operands, out_avals=tuple(out_avals), in_names=tuple(all_in_names),
            out_names=tuple(out_names), lowering_input_output_aliases=(),
            sim_require_finite=False, sim_require_nnan=False, nc=nc)
        return tuple(outs)

    devices = jax.devices()[:N_CORES]
    mesh = Mesh(np.asarray(devices), ("core",))
    specs = (PartitionSpec("core"),) * (n_params + len(out_names))
    fn = jax.jit(shard_map(_body, mesh=mesh, in_specs=specs,
                           out_specs=(PartitionSpec("core"),) * len(out_names),
                           check_rep=False), keep_unused=True)
    return nc, fn, in_names, out_names, out_avals, zero_outs, mesh


def _device_args(p):
    import jax
    from jax.sharding import NamedSharding, PartitionSpec
    nc, fn, in_names, out_names, out_avals, zero_outs, mesh = _CACHE["runner"]
    per_core_maps = []
    for c in range(N_CORES):
        m = dict(p.shared)
        m.update(p.per_core[c])
        per_core_maps.append(m)
    concat_in = [np.concatenate([np.asarray(per_core_maps[c][nm])[None]
                                 for c in range(N_CORES)], axis=0)
                 .reshape(-1, *np.asarray(per_core_maps[0][nm]).shape[1:])
                 for nm in in_names]
    concat_zero = [np.zeros((N_CORES * z.shape[0], *z.shape[1:]), z.dtype)
                   for z in zero_outs]
    sh = NamedSharding(mesh, PartitionSpec("core"))
    args = [jax.device_put(a, sh) for a in concat_in + concat_zero]
    for a in args:
        a.block_until_ready()
    return args


def run_on_device(p):
    import jax
    sig = (p.N, p.E, p.G, p.Ktot, tuple(map(int, p.K_pass)),
           tuple(map(int, p.k_fix.ravel())))
    if _CACHE.get("sig") != sig:
        _CACHE.clear()
        _CACHE["sig"] = sig
    if "runner" not in _CACHE:
        _CACHE["runner"] = _get_runner(p)
    if "args" not in _CACHE:
        _CACHE["args"] = _device_args(p)
    nc, fn, in_names, out_names, out_avals, zero_outs, mesh = _CACHE["runner"]
    outs = fn(*_CACHE["args"])
    for o in outs:
        o.block_until_ready()
    res = np.asarray(outs[out_names.index("out")])
    res = res.reshape(N_CORES, 1, 512)[0, 0]     # core 0
    return res


def kernel(**inputs):
    p = preprocess(**inputs)
    _CACHE.pop("args", None)       # force fresh input upload for new data
    out = run_on_device(p)
    return out[:p.G].astype(np.float32).reshape(p.G, 1)

